# revision 10
# baseline (speedup 1.0000x reference)
"""Causal multi-head attention on 8 Trainium2 NeuronCores (Bass/Tile).

Problem: Q,K,V [B=2, h=16, S=2048, d=64] fp32; out = softmax(QK^T/8, causal) V.

Sharding: B*h = 32 heads split 4-per-core across 8 cores (head-parallel);
each core computes full causal attention for its 4 heads.

v6: pure-matmul PE stream. The TRN2 PE clock is gated by a hardware
activity monitor (HAM): 1.2 GHz cold by default, 2.4 GHz only after
~3.4us of sustained matmul activity, and transpose-mode instructions do
NOT count as activity. Earlier versions interleaved PE transposes
(Q/K/O) with the matmul stream, punching HAM holes that kept the PE
cold half the time. v6 removes every PE transpose:
  - Q,K are cast fp32->bf16 into DRAM staging [S, 128] (SWDGE), then
    loaded transposed via one XBAR DMA-transpose each -> qt/kt [128, S]
    bf16 with the real data on partitions 0:64 (single-half layout, no
    row duplication).
  - The O-tail transposes O^T via SBUF->SBUF XBAR DMA (bf16).
  - Ident-matmul filler (never-read trash PSUM tile, zero semaphores)
    plugs the PE stream wherever ACT exp latency would otherwise stall
    it, keeping HAM warm.
Inner loop (per q-chunk c of 512, per live k-tile pair t):
  S^T pair [128k, 1024] in PSUM via two matmuls (contraction d=64 on
  partitions 0:64), causal -1e30 mask added on diagonal blocks (DVE),
  one merged exp via ACT (scale=1/8) -> P^T bf16 in SBUF, then
  O^T[65, 512] += V'_j^T @ P^T_j in PSUM (row 64 = softmax denominator
  via the ones column of V').
Tail per chunk: O^T -> bf16 SBUF, 4 DMA transposes -> [128q, *] tiles,
batched reciprocal of l, 4 scalar muls, one output DMA.
"""

import numpy as np

import concourse.bass as bass
import concourse.mybir as mybir
import concourse.tile as tile
from concourse.bass_utils import run_bass_kernel_spmd
from concourse.tile import add_dep_helper

N_CORES = 8
B, H, S, D = 2, 16, 2048, 64
HEADS_PER_CORE = (B * H) // N_CORES  # 4
NT = S // 128           # 16 k/q tiles per head
NCHUNK = S // 512       # 4 q-chunks per head
F32 = mybir.dt.float32
F32R = mybir.dt.float32r
BF16 = mybir.dt.bfloat16
NEG = -1.0e30


class SplitDrainTileContext(tile.TileContext):
    """TileContext whose tail drain splits its semaphore waits across
    single-wait SP nops — the TPB CTRL_NO struct holds one wait slot, so
    a drain waiting on >1 proc fails walrus codegen."""

    def _drain_and_barrier(self, tick_clock, wait_clock):
        import bass_rust
        from concourse.vector_clock import ScopedClock

        gc = tick_clock.global_clock
        for i, v in enumerate(list(gc)):
            if v <= 0:
                continue
            c = bass_rust.VectorClock()
            c.require_at_least(i, v)
            nop = self.nc.sync.nop(hint="preDrain", nofuse=True)
            wait_clock.add_sem_waits(nop.ins, ScopedClock({None: c}))
        drain_inst = self.nc.sync.drain()
        wait_clock.add_sem_waits(
            drain_inst.ins, ScopedClock({None: bass_rust.VectorClock()})
        )
        self.nc.all_engine_barrier()
        assert self.sems is not None
        popped = self.nc._tile_sem_poison_stack.pop()
        assert popped is self._sem_poison
        self.nc.clear_and_free_semaphores(list(self.sems.allocated().values()))
        self.nc.all_engine_barrier()


def pe_touch(nc, ap):
    """1-column bf16 ldweights reading `ap` — engine-level PE instruction
    that absorbs a producer's sync wait into the PE engine clock so that
    following 4-byte matmuls need at most one wait (walrus S3_LW limit)."""
    return nc.tensor.ldweights(ap.bitcast(mybir.dt.bfloat16))


def split_waits(nc):
    """Post-pass: every TPB instruction holds exactly ONE sync-wait slot;
    walrus codegen rejects more. Move extra waits onto inserted same-engine
    nofuse nops placed immediately before the instruction."""
    cnt = 0
    for fn in nc.m.functions:
        for bb in fn.blocks:
            lst = bb.instructions
            i = 0
            while i < len(lst):
                ins = lst[i]
                si = ins.sync_info
                if si is not None and si.on_wait and len(si.on_wait) > 1:
                    waits = list(si.on_wait)
                    for w in waits[:-1]:
                        nop = mybir.InstNoOp(name=f"wsplit_{cnt}", ins=[], outs=[])
                        cnt += 1
                        nop.engine = ins.engine
                        nop.bass_nofuse = True
                        nop.sync_info = mybir.SyncInfo(on_wait=[w], on_update=[])
                        lst.insert(i, nop)
                        i += 1
                    si.on_wait = [waits[-1]]
                i += 1
    return cnt


def build_kernel():
    nc = bass.Bass(trn_type="TRN2")
    q_d = nc.dram_tensor("Q", [HEADS_PER_CORE, S, D], F32, kind="ExternalInput")
    k_d = nc.dram_tensor("K", [HEADS_PER_CORE, S, D], F32, kind="ExternalInput")
    v_d = nc.dram_tensor("V", [HEADS_PER_CORE, S, D], F32, kind="ExternalInput")
    o_d = nc.dram_tensor("O", [HEADS_PER_CORE, S, D], F32, kind="ExternalOutput")
    # bf16 staging for XBAR transpose loads (cols 64:128 junk, never read)
    qb_d = nc.dram_tensor("QB", [HEADS_PER_CORE, S, 128], BF16, kind="Internal")
    kb_d = nc.dram_tensor("KB", [HEADS_PER_CORE, S, 128], BF16, kind="Internal")

    with SplitDrainTileContext(nc) as tc:
        import contextlib

        with contextlib.ExitStack() as ctx:
            consts = ctx.enter_context(tc.tile_pool(name="consts", bufs=1))
            v_pool = ctx.enter_context(tc.tile_pool(name="vp", bufs=2))
            qt_pool = ctx.enter_context(tc.tile_pool(name="qt", bufs=2))
            kt_pool = ctx.enter_context(tc.tile_pool(name="kt", bufs=2))
            pt_pool = ctx.enter_context(tc.tile_pool(name="pt", bufs=4))
            otb_pool = ctx.enter_context(tc.tile_pool(name="otb", bufs=3))
            oqb_pool = ctx.enter_context(tc.tile_pool(name="oqb", bufs=3))
            out_pool = ctx.enter_context(tc.tile_pool(name="out", bufs=4))
            r_pool = ctx.enter_context(tc.tile_pool(name="recip", bufs=4))

            trash_ps = ctx.enter_context(
                tc.tile_pool(name="trashps", bufs=1, space="PSUM"))
            st_ps = ctx.enter_context(tc.tile_pool(name="stps", bufs=2, space="PSUM"))
            ot_ps = ctx.enter_context(tc.tile_pool(name="otps", bufs=3, space="PSUM"))

            # constants
            ident_f = consts.tile([128, 128], F32, tag="ident_f")
            nc.gpsimd.memset(ident_f[:], 0.0)
            nc.gpsimd.affine_select(
                out=ident_f[:], in_=ident_f[:],
                compare_op=mybir.AluOpType.not_equal, fill=1.0, base=0,
                pattern=[[-1, 128]], channel_multiplier=1,
            )
            # causal mask for S^T [k_part, q_free] diag blocks:
            # keep 0 where q >= k (f >= p), NEG below
            nmask = consts.tile([128, 128], F32, tag="nmask")
            nc.gpsimd.memset(nmask[:], NEG)
            nc.gpsimd.affine_select(
                out=nmask[:], in_=nmask[:],
                compare_op=mybir.AluOpType.is_gt, fill=0.0, base=0,
                pattern=[[-1, 128]], channel_multiplier=1,
            )
            ones16 = consts.tile([128, NT], F32, tag="ones16")
            nc.gpsimd.memset(ones16[:], 1.0)
            t_if = pe_touch(nc, ident_f[0:1, 0:1])
            trash = trash_ps.tile([128, 512], F32, tag="trash")

            def emit_fill(n=1):
                # zero-semaphore PE filler: keeps the HAM activity monitor
                # warm while ACT drains the exp backlog
                for _ in range(2 * n):
                    nc.tensor.matmul(
                        trash[:, 0:256],
                        ident_f[:, 0:64].bitcast(BF16),
                        ident_f[:, 0:128].bitcast(BF16),
                        start=True, stop=True, skip_group_check=True,
                    )

            for _ in range(12):
                emit_fill()

            # ---------------- prep: stage-cast + transpose loads ------------
            def prep_loads(h, state):
                # SWDGE cast fp32 -> bf16 into DRAM staging (cols 0:64)
                nc.gpsimd.dma_start(qb_d[h][:, 0:64], q_d[h])
                nc.gpsimd.dma_start(kb_d[h][:, 0:64], k_d[h])
                vp = v_pool.tile([128, NT * 65], BF16, tag="vp")
                vp3 = vp[:].rearrange("p (t e) -> p t e", e=65)
                nc.gpsimd.dma_start(
                    vp3[:, :, 0:64],
                    v_d[h].rearrange("(t p) d -> p t d", p=128),
                )
                nc.vector.tensor_copy(vp3[:, :, 64:65], ones16[:])
                state["vp"] = vp
                state["tv"] = [
                    pe_touch(nc, vp[0:1, 0:1]),
                    pe_touch(nc, vp[0:1, 64:65]),
                ]

            def prep_tr(h, state):
                # one XBAR DMA-transpose per tensor: [S, 128] bf16 DRAM ->
                # [128, S] bf16 SBUF; real data on partitions 0:64
                qt = qt_pool.tile([128, S], BF16, tag="qt")
                nc.sync.dma_start(qt[:], qb_d[h], transpose=True)
                kt = kt_pool.tile([128, S], BF16, tag="kt")
                nc.sync.dma_start(kt[:], kb_d[h], transpose=True)
                state["qt"], state["kt"] = qt, kt

            def prep_finish(state):
                state["tq1"] = pe_touch(nc, state["qt"][0:1, 0:1])
                state["tk1"] = pe_touch(nc, state["kt"][0:1, 0:1])
                state["first_qk"] = True

            def prep_all(h, state):
                prep_loads(h, state)
                prep_tr(h, state)
                prep_finish(state)

            # ---------------- pipelined pair units -------------------------
            # unit = (h, c, t); chunk c has npair = 2c+2 live pairs; pair t
            # covers k-tiles j1 = 2t, j2 = 2t+1; diagonal pairs are t = 2c
            # (cA=0, cB=128) and t = 2c+1 (cA=256, cB=384).

            def emit_qk_exp(u, states):
                h, c, t = u
                state = states[h]
                qt, kt = state["qt"], state["kt"]
                j1, j2 = 2 * t, 2 * t + 1
                cA = 128 * j1 - 512 * c
                cB = 128 * j2 - 512 * c
                cA0 = max(0, cA)
                cB0 = max(0, cB)
                st = st_ps.tile([128, 1024], F32, tag="st")
                mmA = nc.tensor.matmul(
                    st[:, cA0:512],
                    kt[0:64, 128 * j1:128 * j1 + 128],
                    qt[0:64, 512 * c + cA0:512 * c + 512],
                    start=True, stop=True,
                )
                if state.pop("first_qk", False):
                    for tch in (state["tq1"], state["tk1"], *state["tv"]):
                        add_dep_helper(mmA.ins, tch.ins, sync=False,
                                       reason="presync")
                nc.tensor.matmul(
                    st[:, 512 + cB0:1024],
                    kt[0:64, 128 * j2:128 * j2 + 128],
                    qt[0:64, 512 * c + cB0:512 * c + 512],
                    start=True, stop=True,
                )
                # causal masks on the diagonal blocks (DVE, pre-exp)
                if j1 >= 4 * c:
                    nc.vector.tensor_tensor(
                        st[:, cA:cA + 128], st[:, cA:cA + 128],
                        nmask[:], mybir.AluOpType.add,
                    )
                if j2 >= 4 * c:
                    nc.vector.tensor_tensor(
                        st[:, 512 + cB:512 + cB + 128],
                        st[:, 512 + cB:512 + cB + 128],
                        nmask[:], mybir.AluOpType.add,
                    )
                # one merged exp per pair; the [512:512+cB0] hole reads
                # stale PSUM whose exp lands in pt cols PV never touches
                pt = pt_pool.tile([128, 1024], BF16, tag="pt")
                nc.scalar.activation(
                    pt[:, cA0:1024], st[:, cA0:1024],
                    mybir.ActivationFunctionType.Exp, scale=0.125,
                )
                return {"pt": pt, "u": u}

            def emit_pv(unit_data, states, ot_map):
                h, c, t = unit_data["u"]
                state = states[h]
                vp = state["vp"]
                pt = unit_data["pt"]
                npair = min(2 * c + 2, 8)
                j1, j2 = 2 * t, 2 * t + 1
                vA = max(0, 128 * j1 - 512 * c)
                vB = max(0, 128 * j2 - 512 * c)
                if t == 0:
                    ot_map[(h, c)] = ot_ps.tile(
                        [65, 512], F32, tag="ot", name="ot"
                    )
                ot = ot_map[(h, c)]
                nc.tensor.matmul(
                    ot[:, vA:512],
                    vp[:, 65 * j1:65 * j1 + 65],
                    pt[:, vA:512],
                    start=(t == 0), stop=False,
                    skip_group_check=True,
                )
                nc.tensor.matmul(
                    ot[:, vB:512],
                    vp[:, 65 * j2:65 * j2 + 65],
                    pt[:, 512 + vB:1024],
                    start=False, stop=(t == npair - 1),
                    skip_group_check=True,
                )

            def emit_tail(h, c, ot_map):
                ot = ot_map.pop((h, c))
                otb = otb_pool.tile([96, 512], BF16, tag="otb")
                nc.vector.tensor_copy(otb[0:65, :], ot[:, :])
                oqb = oqb_pool.tile([128, 4 * 96], BF16, tag="oqb")
                for i in range(4):
                    nc.sync.dma_start(
                        oqb[:, 96 * i:96 * i + 96],
                        otb[0:96, 128 * i:128 * i + 128],
                        transpose=True,
                    )
                oqv = oqb[:].rearrange("p (i w) -> p i w", w=96)
                rec = r_pool.tile([128, 4], F32, tag="rec")
                nc.vector.reciprocal(
                    rec[:].rearrange("p (i o) -> p i o", o=1),
                    oqv[:, :, 64:65],
                )
                ob = out_pool.tile([128, 256], F32, tag="ob")
                for i in range(4):
                    nc.vector.tensor_scalar_mul(
                        ob[:, 64 * i:64 * i + 64],
                        oqb[:, 96 * i:96 * i + 64],
                        rec[:, i:i + 1],
                    )
                nc.sync.dma_start(
                    o_d[h].rearrange("(t p) d -> p t d", p=128)[:, 4*c:4*c+4, :],
                    ob[:].rearrange("p (t d) -> p t d", d=64),
                )

            # HAM-warming filler counts per (c, t)
            def dead_plan(h, c):
                live = min(2 * c + 2, 8)
                extra = {0: 1, 1: 2, 2: 2, 3: 3}[c]
                plan = [1] * live
                for d in range(extra):
                    plan[(d * live) // max(extra, 1) % live] += 1
                return plan

            fillers = {}
            for h in range(HEADS_PER_CORE - 1):
                nh = h + 1
                fillers[(h, 2, 0)] = [lambda s, _h=nh: prep_loads(_h, s[_h])]
                fillers[(h, 3, 0)] = [lambda s, _h=nh: prep_tr(_h, s[_h])]
                fillers[(h, 3, 1)] = [lambda s, _h=nh: prep_finish(s[_h])]

            units = []
            for h in range(HEADS_PER_CORE):
                for c in range(NCHUNK):
                    for t in range(min(2 * c + 2, 8)):
                        units.append((h, c, t))

            states = {h: {} for h in range(HEADS_PER_CORE)}
            prep_all(0, states[0])

            ot_map = {}
            lag = []  # pending units awaiting PV emission
            for u in units:
                if len(lag) == 2:
                    ud = lag.pop(0)
                    emit_pv(ud, states, ot_map)
                    _h, _c, _t = ud["u"]
                    if _t == min(2 * _c + 2, 8) - 1:
                        emit_tail(_h, _c, ot_map)
                for f in fillers.get(u, []):
                    f(states)
                lag.append(emit_qk_exp(u, states))
                _h, _c, _t = u
                plan = dead_plan(_h, _c)
                emit_fill(plan[_t])
            while lag:
                ud = lag.pop(0)
                emit_pv(ud, states, ot_map)
                _h, _c, _t = ud["u"]
                if _t == min(2 * _c + 2, 8) - 1:
                    emit_tail(_h, _c, ot_map)

    split_waits(nc)
    return nc


_CACHED = {}


def kernel(Q: np.ndarray, K: np.ndarray, V: np.ndarray) -> np.ndarray:
    res = _run(Q, K, V, trace=False)
    return res[0]


def _run(Q, K, V, trace=False):
    Qf = np.ascontiguousarray(Q.reshape(B * H, S, D), dtype=np.float32)
    Kf = np.ascontiguousarray(K.reshape(B * H, S, D), dtype=np.float32)
    Vf = np.ascontiguousarray(V.reshape(B * H, S, D), dtype=np.float32)

    in_maps = []
    for c in range(N_CORES):
        sl = slice(c * HEADS_PER_CORE, (c + 1) * HEADS_PER_CORE)
        in_maps.append({
            "Q": np.ascontiguousarray(Qf[sl]),
            "K": np.ascontiguousarray(Kf[sl]),
            "V": np.ascontiguousarray(Vf[sl]),
        })

    if "nc" not in _CACHED:
        _CACHED["nc"] = build_kernel()
    nc = _CACHED["nc"]

    res = run_bass_kernel_spmd(
        nc, in_maps, core_ids=list(range(N_CORES)), trace=trace
    )
    out = np.empty((B * H, S, D), dtype=np.float32)
    for c in range(N_CORES):
        out[c * HEADS_PER_CORE:(c + 1) * HEADS_PER_CORE] = res.results[c]["O"]
    return out.reshape(B, H, S, D), res


# revision 11
# speedup vs baseline: 1.1648x; 1.1648x over previous
"""Causal multi-head attention on 8 Trainium2 NeuronCores (Bass/Tile).

Problem: Q,K,V [B=2, h=16, S=2048, d=64] fp32; out = softmax(QK^T/8, causal) V.

Sharding: B*h = 32 heads split 4-per-core across 8 cores (head-parallel);
each core computes full causal attention for its 4 heads.

v6: pure-matmul PE stream. The TRN2 PE clock is gated by a hardware
activity monitor (HAM): 1.2 GHz cold by default, 2.4 GHz only after
~3.4us of sustained matmul activity, and transpose-mode instructions do
NOT count as activity. Earlier versions interleaved PE transposes
(Q/K/O) with the matmul stream, punching HAM holes that kept the PE
cold half the time. v6 removes every PE transpose:
  - Q,K are cast fp32->bf16 into DRAM staging [S, 128] (SWDGE), then
    loaded transposed via one XBAR DMA-transpose each -> qt/kt [128, S]
    bf16 with the real data on partitions 0:64 (single-half layout, no
    row duplication).
  - The O-tail transposes O^T via SBUF->SBUF XBAR DMA (bf16).
  - Ident-matmul filler (never-read trash PSUM tile, zero semaphores)
    plugs the PE stream wherever ACT exp latency would otherwise stall
    it, keeping HAM warm.
Inner loop (per q-chunk c of 512, per live k-tile pair t):
  S^T pair [128k, 1024] in PSUM via two matmuls (contraction d=64 on
  partitions 0:64), causal -1e30 mask added on diagonal blocks (DVE),
  one merged exp via ACT (scale=1/8) -> P^T bf16 in SBUF, then
  O^T[65, 512] += V'_j^T @ P^T_j in PSUM (row 64 = softmax denominator
  via the ones column of V').
Tail per chunk: O^T -> bf16 SBUF, 4 DMA transposes -> [128q, *] tiles,
batched reciprocal of l, 4 scalar muls, one output DMA.
"""

import numpy as np

import concourse.bass as bass
import concourse.mybir as mybir
import concourse.tile as tile
from concourse.bass_utils import run_bass_kernel_spmd
from concourse.tile import add_dep_helper

N_CORES = 8
B, H, S, D = 2, 16, 2048, 64
HEADS_PER_CORE = (B * H) // N_CORES  # 4
NT = S // 128           # 16 k/q tiles per head
NCHUNK = S // 512       # 4 q-chunks per head
F32 = mybir.dt.float32
F32R = mybir.dt.float32r
BF16 = mybir.dt.bfloat16
NEG = -1.0e30


class SplitDrainTileContext(tile.TileContext):
    """TileContext whose tail drain splits its semaphore waits across
    single-wait SP nops — the TPB CTRL_NO struct holds one wait slot, so
    a drain waiting on >1 proc fails walrus codegen."""

    def _drain_and_barrier(self, tick_clock, wait_clock):
        import bass_rust
        from concourse.vector_clock import ScopedClock

        gc = tick_clock.global_clock
        for i, v in enumerate(list(gc)):
            if v <= 0:
                continue
            c = bass_rust.VectorClock()
            c.require_at_least(i, v)
            nop = self.nc.sync.nop(hint="preDrain", nofuse=True)
            wait_clock.add_sem_waits(nop.ins, ScopedClock({None: c}))
        drain_inst = self.nc.sync.drain()
        wait_clock.add_sem_waits(
            drain_inst.ins, ScopedClock({None: bass_rust.VectorClock()})
        )
        self.nc.all_engine_barrier()
        assert self.sems is not None
        popped = self.nc._tile_sem_poison_stack.pop()
        assert popped is self._sem_poison
        self.nc.clear_and_free_semaphores(list(self.sems.allocated().values()))
        self.nc.all_engine_barrier()


def pe_touch(nc, ap):
    """1-column bf16 ldweights reading `ap` — engine-level PE instruction
    that absorbs a producer's sync wait into the PE engine clock so that
    following 4-byte matmuls need at most one wait (walrus S3_LW limit)."""
    return nc.tensor.ldweights(ap.bitcast(mybir.dt.bfloat16))


def split_waits(nc):
    """Post-pass: every TPB instruction holds exactly ONE sync-wait slot;
    walrus codegen rejects more. Move extra waits onto inserted same-engine
    nofuse nops placed immediately before the instruction."""
    cnt = 0
    for fn in nc.m.functions:
        for bb in fn.blocks:
            lst = bb.instructions
            i = 0
            while i < len(lst):
                ins = lst[i]
                si = ins.sync_info
                if si is not None and si.on_wait and len(si.on_wait) > 1:
                    waits = list(si.on_wait)
                    for w in waits[:-1]:
                        nop = mybir.InstNoOp(name=f"wsplit_{cnt}", ins=[], outs=[])
                        cnt += 1
                        nop.engine = ins.engine
                        nop.bass_nofuse = True
                        nop.sync_info = mybir.SyncInfo(on_wait=[w], on_update=[])
                        lst.insert(i, nop)
                        i += 1
                    si.on_wait = [waits[-1]]
                i += 1
    return cnt


def build_kernel():
    nc = bass.Bass(trn_type="TRN2")
    q_d = nc.dram_tensor("Q", [HEADS_PER_CORE, S, D], F32, kind="ExternalInput")
    k_d = nc.dram_tensor("K", [HEADS_PER_CORE, S, D], F32, kind="ExternalInput")
    v_d = nc.dram_tensor("V", [HEADS_PER_CORE, S, D], F32, kind="ExternalInput")
    o_d = nc.dram_tensor("O", [HEADS_PER_CORE, S, D], F32, kind="ExternalOutput")
    # bf16 staging for XBAR transpose loads (cols 64:128 junk, never read)
    qb_d = nc.dram_tensor("QB", [HEADS_PER_CORE, S, 128], BF16, kind="Internal")
    kb_d = nc.dram_tensor("KB", [HEADS_PER_CORE, S, 128], BF16, kind="Internal")

    with SplitDrainTileContext(nc) as tc:
        import contextlib

        with contextlib.ExitStack() as ctx:
            consts = ctx.enter_context(tc.tile_pool(name="consts", bufs=1))
            v_pool = ctx.enter_context(tc.tile_pool(name="vp", bufs=2))
            qt_pool = ctx.enter_context(tc.tile_pool(name="qt", bufs=2))
            kt_pool = ctx.enter_context(tc.tile_pool(name="kt", bufs=2))
            pt_pool = ctx.enter_context(tc.tile_pool(name="pt", bufs=4))
            otb_pool = ctx.enter_context(tc.tile_pool(name="otb", bufs=3))
            out_pool = ctx.enter_context(tc.tile_pool(name="out", bufs=4))
            r_pool = ctx.enter_context(tc.tile_pool(name="recip", bufs=4))

            trash_ps = ctx.enter_context(
                tc.tile_pool(name="trashps", bufs=1, space="PSUM"))
            st_ps = ctx.enter_context(tc.tile_pool(name="stps", bufs=2, space="PSUM"))
            ot_ps = ctx.enter_context(tc.tile_pool(name="otps", bufs=2, space="PSUM"))
            tr_ps = ctx.enter_context(tc.tile_pool(name="trps", bufs=1, space="PSUM"))

            # constants
            ident_f = consts.tile([128, 128], F32, tag="ident_f")
            nc.gpsimd.memset(ident_f[:], 0.0)
            nc.gpsimd.affine_select(
                out=ident_f[:], in_=ident_f[:],
                compare_op=mybir.AluOpType.not_equal, fill=1.0, base=0,
                pattern=[[-1, 128]], channel_multiplier=1,
            )
            # causal mask for S^T [k_part, q_free] diag blocks:
            # keep 0 where q >= k (f >= p), NEG below
            nmask = consts.tile([128, 128], F32, tag="nmask")
            nc.gpsimd.memset(nmask[:], NEG)
            nc.gpsimd.affine_select(
                out=nmask[:], in_=nmask[:],
                compare_op=mybir.AluOpType.is_gt, fill=0.0, base=0,
                pattern=[[-1, 128]], channel_multiplier=1,
            )
            ident_b = consts.tile([128, 128], BF16, tag="ident_b")
            nc.vector.tensor_copy(ident_b[:], ident_f[:])
            ones16 = consts.tile([128, NT], F32, tag="ones16")
            nc.gpsimd.memset(ones16[:], 1.0)
            t_if = pe_touch(nc, ident_f[0:1, 0:1])
            trash = trash_ps.tile([128, 512], F32, tag="trash")

            def emit_fill(n=1):
                # zero-semaphore PE filler: keeps the HAM activity monitor
                # warm while ACT drains the exp backlog
                for _ in range(2 * n):
                    nc.tensor.matmul(
                        trash[:, 0:256],
                        ident_f[:, 0:64].bitcast(BF16),
                        ident_f[:, 0:128].bitcast(BF16),
                        start=True, stop=True, skip_group_check=True,
                    )

            for _ in range(12):
                emit_fill()

            # ---------------- prep: stage-cast + transpose loads ------------
            def prep_loads(h, state):
                # SWDGE cast fp32 -> bf16 into DRAM staging (cols 0:64)
                nc.gpsimd.dma_start(qb_d[h][:, 0:64], q_d[h])
                nc.gpsimd.dma_start(kb_d[h][:, 0:64], k_d[h])
                vp = v_pool.tile([128, NT * 65], BF16, tag="vp")
                vp3 = vp[:].rearrange("p (t e) -> p t e", e=65)
                nc.gpsimd.dma_start(
                    vp3[:, :, 0:64],
                    v_d[h].rearrange("(t p) d -> p t d", p=128),
                )
                nc.vector.tensor_copy(vp3[:, :, 64:65], ones16[:])
                state["vp"] = vp
                state["tv"] = [
                    pe_touch(nc, vp[0:1, 0:1]),
                    pe_touch(nc, vp[0:1, 64:65]),
                ]

            def prep_tr(h, state):
                # one XBAR DMA-transpose per tensor: [S, 128] bf16 DRAM ->
                # [128, S] bf16 SBUF; real data on partitions 0:64
                qt = qt_pool.tile([128, S], BF16, tag="qt")
                nc.sync.dma_start(qt[:], qb_d[h], transpose=True)
                kt = kt_pool.tile([128, S], BF16, tag="kt")
                nc.sync.dma_start(kt[:], kb_d[h], transpose=True)
                state["qt"], state["kt"] = qt, kt

            def prep_finish(state):
                state["tq1"] = pe_touch(nc, state["qt"][0:1, 0:1])
                state["tk1"] = pe_touch(nc, state["kt"][0:1, 0:1])
                state["first_qk"] = True

            def prep_all(h, state):
                prep_loads(h, state)
                prep_tr(h, state)
                prep_finish(state)

            # ---------------- pipelined pair units -------------------------
            # unit = (h, c, t); chunk c has npair = 2c+2 live pairs; pair t
            # covers k-tiles j1 = 2t, j2 = 2t+1; diagonal pairs are t = 2c
            # (cA=0, cB=128) and t = 2c+1 (cA=256, cB=384).

            def emit_qk_exp(u, states):
                h, c, t = u
                state = states[h]
                qt, kt = state["qt"], state["kt"]
                j1, j2 = 2 * t, 2 * t + 1
                cA = 128 * j1 - 512 * c
                cB = 128 * j2 - 512 * c
                cA0 = max(0, cA)
                cB0 = max(0, cB)
                st = st_ps.tile([128, 1024], F32, tag="st")
                mmA = nc.tensor.matmul(
                    st[:, cA0:512],
                    kt[0:64, 128 * j1:128 * j1 + 128],
                    qt[0:64, 512 * c + cA0:512 * c + 512],
                    start=True, stop=True,
                )
                if state.pop("first_qk", False):
                    for tch in (state["tq1"], state["tk1"], *state["tv"]):
                        add_dep_helper(mmA.ins, tch.ins, sync=False,
                                       reason="presync")
                nc.tensor.matmul(
                    st[:, 512 + cB0:1024],
                    kt[0:64, 128 * j2:128 * j2 + 128],
                    qt[0:64, 512 * c + cB0:512 * c + 512],
                    start=True, stop=True,
                )
                # causal masks on the diagonal blocks (DVE, pre-exp)
                if j1 >= 4 * c:
                    nc.vector.tensor_tensor(
                        st[:, cA:cA + 128], st[:, cA:cA + 128],
                        nmask[:], mybir.AluOpType.add,
                    )
                if j2 >= 4 * c:
                    nc.vector.tensor_tensor(
                        st[:, 512 + cB:512 + cB + 128],
                        st[:, 512 + cB:512 + cB + 128],
                        nmask[:], mybir.AluOpType.add,
                    )
                # one merged exp per pair; the [512:512+cB0] hole reads
                # stale PSUM whose exp lands in pt cols PV never touches
                pt = pt_pool.tile([128, 1024], BF16, tag="pt")
                nc.scalar.activation(
                    pt[:, cA0:1024], st[:, cA0:1024],
                    mybir.ActivationFunctionType.Exp, scale=0.125,
                )
                return {"pt": pt, "u": u}

            def emit_pv(unit_data, states, ot_map):
                h, c, t = unit_data["u"]
                state = states[h]
                vp = state["vp"]
                pt = unit_data["pt"]
                npair = min(2 * c + 2, 8)
                j1, j2 = 2 * t, 2 * t + 1
                vA = max(0, 128 * j1 - 512 * c)
                vB = max(0, 128 * j2 - 512 * c)
                if t == 0:
                    ot_map[(h, c)] = ot_ps.tile(
                        [65, 512], F32, tag="ot", name="ot"
                    )
                ot = ot_map[(h, c)]
                nc.tensor.matmul(
                    ot[:, vA:512],
                    vp[:, 65 * j1:65 * j1 + 65],
                    pt[:, vA:512],
                    start=(t == 0), stop=False,
                    skip_group_check=True,
                )
                nc.tensor.matmul(
                    ot[:, vB:512],
                    vp[:, 65 * j2:65 * j2 + 65],
                    pt[:, 512 + vB:1024],
                    start=False, stop=(t == npair - 1),
                    skip_group_check=True,
                )

            def emit_tail(h, c, ot_map):
                ot = ot_map.pop((h, c))
                otb = otb_pool.tile([65, 512], BF16, tag="otb")
                nc.vector.tensor_copy(otb[:, :], ot[:, :])
                oqb = tr_ps.tile([128, 4 * 96], BF16, tag="oq", name="oqb")
                for i in range(4):
                    nc.tensor.transpose(
                        oqb[:, 96 * i:96 * i + 96],
                        otb[0:65, 128 * i:128 * i + 128],
                        ident_b[0:65, 0:96],
                    )
                oqv = oqb[:].rearrange("p (i w) -> p i w", w=96)
                rec = r_pool.tile([128, 4], F32, tag="rec")
                nc.vector.reciprocal(
                    rec[:].rearrange("p (i o) -> p i o", o=1),
                    oqv[:, :, 64:65],
                )
                ob = out_pool.tile([128, 256], F32, tag="ob")
                for i in range(4):
                    nc.vector.tensor_scalar_mul(
                        ob[:, 64 * i:64 * i + 64],
                        oqb[:, 96 * i:96 * i + 64],
                        rec[:, i:i + 1],
                    )
                nc.sync.dma_start(
                    o_d[h].rearrange("(t p) d -> p t d", p=128)[:, 4*c:4*c+4, :],
                    ob[:].rearrange("p (t d) -> p t d", d=64),
                )

            # HAM-warming filler counts per (c, t)
            def dead_plan(h, c):
                live = min(2 * c + 2, 8)
                extra = {0: 1, 1: 2, 2: 2, 3: 3}[c]
                plan = [1] * live
                for d in range(extra):
                    plan[(d * live) // max(extra, 1) % live] += 1
                return plan

            fillers = {}
            for h in range(HEADS_PER_CORE - 1):
                nh = h + 1
                fillers[(h, 1, 0)] = [lambda s, _h=nh: prep_loads(_h, s[_h])]
                fillers[(h, 2, 0)] = [lambda s, _h=nh: prep_tr(_h, s[_h])]
                fillers[(h, 3, 0)] = [lambda s, _h=nh: prep_finish(s[_h])]

            units = []
            for h in range(HEADS_PER_CORE):
                for c in range(NCHUNK):
                    for t in range(min(2 * c + 2, 8)):
                        units.append((h, c, t))

            states = {h: {} for h in range(HEADS_PER_CORE)}
            prep_all(0, states[0])

            ot_map = {}
            lag = []  # pending units awaiting PV emission
            for u in units:
                if len(lag) == 2:
                    ud = lag.pop(0)
                    emit_pv(ud, states, ot_map)
                    _h, _c, _t = ud["u"]
                    if _t == min(2 * _c + 2, 8) - 1:
                        emit_tail(_h, _c, ot_map)
                for f in fillers.get(u, []):
                    f(states)
                lag.append(emit_qk_exp(u, states))
                _h, _c, _t = u
                plan = dead_plan(_h, _c)
                emit_fill(plan[_t])
            while lag:
                ud = lag.pop(0)
                emit_pv(ud, states, ot_map)
                _h, _c, _t = ud["u"]
                if _t == min(2 * _c + 2, 8) - 1:
                    emit_tail(_h, _c, ot_map)

    split_waits(nc)
    return nc


_CACHED = {}


def kernel(Q: np.ndarray, K: np.ndarray, V: np.ndarray) -> np.ndarray:
    res = _run(Q, K, V, trace=False)
    return res[0]


def _run(Q, K, V, trace=False):
    Qf = np.ascontiguousarray(Q.reshape(B * H, S, D), dtype=np.float32)
    Kf = np.ascontiguousarray(K.reshape(B * H, S, D), dtype=np.float32)
    Vf = np.ascontiguousarray(V.reshape(B * H, S, D), dtype=np.float32)

    in_maps = []
    for c in range(N_CORES):
        sl = slice(c * HEADS_PER_CORE, (c + 1) * HEADS_PER_CORE)
        in_maps.append({
            "Q": np.ascontiguousarray(Qf[sl]),
            "K": np.ascontiguousarray(Kf[sl]),
            "V": np.ascontiguousarray(Vf[sl]),
        })

    if "nc" not in _CACHED:
        _CACHED["nc"] = build_kernel()
    nc = _CACHED["nc"]

    res = run_bass_kernel_spmd(
        nc, in_maps, core_ids=list(range(N_CORES)), trace=trace
    )
    out = np.empty((B * H, S, D), dtype=np.float32)
    for c in range(N_CORES):
        out[c * HEADS_PER_CORE:(c + 1) * HEADS_PER_CORE] = res.results[c]["O"]
    return out.reshape(B, H, S, D), res


# revision 12
# speedup vs baseline: 1.9011x; 1.6322x over previous
"""Causal multi-head attention on 8 Trainium2 NeuronCores (Bass/Tile).

Problem: Q,K,V [B=2, h=16, S=2048, d=64] fp32; out = softmax(QK^T/8, causal) V.

Sharding: B*h = 32 heads split 4-per-core across 8 cores (head-parallel);
each core computes full causal attention for its 4 heads.

v2: software-pipelined pair loop. The v1 kernel emitted QK(t), mask, exp,
PV(t) per k-tile pair in program order, so the PE sat in a serialized
QK -> DVE mask -> ACT exp -> PV dependency loop (~2.2us/pair). v2 emits
PV(t) after QK(t+1) in the PE stream (distance-1 software pipeline), so
exp(t) on ACT overlaps QK(t+1) on PE and the pace is set by whichever
engine is busiest (ACT exp ~1.05us/pair) instead of the dependency loop.
Other changes: one merged exp per pair (stale PSUM holes are exp'd but
never read), prep(h+1) sliced across the last chunk of head h as PE
filler, batched O-tail (one oq tile, batched reciprocal, one out DMA).

Per-head schedule (all matmuls in float32r — full PE rate at N>=256):
  - Load Q,K natural [128, 16*64] via SWDGE cast fp32->fp32r; V' = [V | 1]
    per k-tile ([128, 16*65]).
  - PE transpose-mode (fp32r): Q -> Q^T [64, 2048] (+DMA row-dup to 64:128),
    K -> K^T stacked pairs [128, 8*128] (k-tile 2t on partitions 0:64,
    2t+1 on 64:128).
  - For each q-chunk c (512 cols) and live k-tile pair t:
      S^T pair [128k, 1024] in PSUM via two row-group matmuls
      (contraction d=64 on partition halves), causal -1e30 mask added on
      the diagonal blocks (DVE), one exp via ACT (scale=1/8 folded in)
      -> P^T [128, 1024] fp32r in SBUF.
      PV: O^T[65, 512] += V'_j^T @ P^T_j accumulated over j in PSUM
      (row 64 = softmax denominator l via the ones column).
  - Per chunk: O^T -> SBUF, 4 exact fp32 transposes via matmul-vs-identity
    into one PSUM tile, batched reciprocal of l, 4 scalar muls, one out DMA.
"""

import numpy as np

import concourse.bass as bass
import concourse.mybir as mybir
import concourse.tile as tile
from concourse.bass_utils import run_bass_kernel_spmd
from concourse.tile import add_dep_helper

N_CORES = 8
B, H, S, D = 2, 16, 2048, 64
HEADS_PER_CORE = (B * H) // N_CORES  # 4
NT = S // 128           # 16 k/q tiles per head
NCHUNK = S // 512       # 4 q-chunks per head
F32 = mybir.dt.float32
F32R = mybir.dt.float32r
BF16 = mybir.dt.bfloat16
NEG = -1.0e30


class SplitDrainTileContext(tile.TileContext):
    """TileContext whose tail drain splits its semaphore waits across
    single-wait SP nops — the TPB CTRL_NO struct holds one wait slot, so
    a drain waiting on >1 proc fails walrus codegen."""

    def _drain_and_barrier(self, tick_clock, wait_clock):
        import bass_rust
        from concourse.vector_clock import ScopedClock

        gc = tick_clock.global_clock
        for i, v in enumerate(list(gc)):
            if v <= 0:
                continue
            c = bass_rust.VectorClock()
            c.require_at_least(i, v)
            nop = self.nc.sync.nop(hint="preDrain", nofuse=True)
            wait_clock.add_sem_waits(nop.ins, ScopedClock({None: c}))
        drain_inst = self.nc.sync.drain()
        wait_clock.add_sem_waits(
            drain_inst.ins, ScopedClock({None: bass_rust.VectorClock()})
        )
        self.nc.all_engine_barrier()
        assert self.sems is not None
        popped = self.nc._tile_sem_poison_stack.pop()
        assert popped is self._sem_poison
        self.nc.clear_and_free_semaphores(list(self.sems.allocated().values()))
        self.nc.all_engine_barrier()


def pe_touch(nc, ap):
    """1-column bf16 ldweights reading `ap` — engine-level PE instruction
    that absorbs a producer's sync wait into the PE engine clock so that
    following 4-byte matmuls need at most one wait (walrus S3_LW limit)."""
    return nc.tensor.ldweights(ap.bitcast(mybir.dt.bfloat16))


def split_waits(nc):
    """Post-pass: every TPB instruction holds exactly ONE sync-wait slot;
    walrus codegen rejects more. Move extra waits onto inserted same-engine
    nofuse nops placed immediately before the instruction."""
    cnt = 0
    for fn in nc.m.functions:
        for bb in fn.blocks:
            lst = bb.instructions
            i = 0
            while i < len(lst):
                ins = lst[i]
                si = ins.sync_info
                if si is not None and si.on_wait and len(si.on_wait) > 1:
                    waits = list(si.on_wait)
                    for w in waits[:-1]:
                        nop = mybir.InstNoOp(name=f"wsplit_{cnt}", ins=[], outs=[])
                        cnt += 1
                        nop.engine = ins.engine
                        nop.bass_nofuse = True
                        nop.sync_info = mybir.SyncInfo(on_wait=[w], on_update=[])
                        lst.insert(i, nop)
                        i += 1
                    si.on_wait = [waits[-1]]
                i += 1
    return cnt


def build_kernel():
    nc = bass.Bass(trn_type="TRN2")
    q_d = nc.dram_tensor("Q", [HEADS_PER_CORE, S, D], F32, kind="ExternalInput")
    k_d = nc.dram_tensor("K", [HEADS_PER_CORE, S, D], F32, kind="ExternalInput")
    v_d = nc.dram_tensor("V", [HEADS_PER_CORE, S, D], F32, kind="ExternalInput")
    o_d = nc.dram_tensor("O", [HEADS_PER_CORE, S, D], F32, kind="ExternalOutput")

    with SplitDrainTileContext(nc) as tc:
        import contextlib

        with contextlib.ExitStack() as ctx:
            consts = ctx.enter_context(tc.tile_pool(name="consts", bufs=1))
            in_pool = ctx.enter_context(tc.tile_pool(name="in", bufs=3))
            v_pool = ctx.enter_context(tc.tile_pool(name="vp", bufs=2))
            qt_pool = ctx.enter_context(tc.tile_pool(name="qt", bufs=2))
            kt_pool = ctx.enter_context(tc.tile_pool(name="kt", bufs=2))
            pt_pool = ctx.enter_context(tc.tile_pool(name="pt", bufs=6))
            otsb_pool = ctx.enter_context(tc.tile_pool(name="otsb", bufs=3))
            out_pool = ctx.enter_context(tc.tile_pool(name="out", bufs=4))
            r_pool = ctx.enter_context(tc.tile_pool(name="recip", bufs=4))

            tr_ps = ctx.enter_context(tc.tile_pool(name="trps", bufs=1, space="PSUM"))
            trash_ps = ctx.enter_context(tc.tile_pool(name="trashps", bufs=1, space="PSUM"))
            st_ps = ctx.enter_context(tc.tile_pool(name="stps", bufs=2, space="PSUM"))
            ot_ps = ctx.enter_context(tc.tile_pool(name="otps", bufs=2, space="PSUM"))

            # constants
            ident_f = consts.tile([128, 128], F32, tag="ident_f")
            nc.gpsimd.memset(ident_f[:], 0.0)
            nc.gpsimd.affine_select(
                out=ident_f[:], in_=ident_f[:],
                compare_op=mybir.AluOpType.not_equal, fill=1.0, base=0,
                pattern=[[-1, 128]], channel_multiplier=1,
            )
            ident_r = consts.tile([128, 128], F32R, tag="ident_r")
            nc.vector.tensor_copy(ident_r[:], ident_f[:])
            ident_b = consts.tile([128, 128], BF16, tag="ident_b")
            nc.vector.tensor_copy(ident_b[:], ident_f[:])
            # causal mask for S^T [k_part, q_free] diag blocks:
            # keep 0 where q >= k (f >= p), NEG below
            nmask = consts.tile([128, 128], F32, tag="nmask")
            nc.gpsimd.memset(nmask[:], NEG)
            nc.gpsimd.affine_select(
                out=nmask[:], in_=nmask[:],
                compare_op=mybir.AluOpType.is_gt, fill=0.0, base=0,
                pattern=[[-1, 128]], channel_multiplier=1,
            )
            ones16 = consts.tile([128, NT], F32, tag="ones16")
            nc.gpsimd.memset(ones16[:], 1.0)
            t_if = pe_touch(nc, ident_f[0:1, 0:1])
            t_ir = pe_touch(nc, ident_r[0:1, 0:1])
            trash = trash_ps.tile([128, 512], F32, tag="trash")
            for _ in range(24):
                nc.tensor.matmul(
                    trash[:, 0:256],
                    ident_f[:, 0:64].bitcast(mybir.dt.bfloat16),
                    ident_f[:, 0:128].bitcast(mybir.dt.bfloat16),
                    start=True, stop=True, skip_group_check=True,
                )

            # ---------------- prep: loads + transposes, sliceable ----------
            def prep_loads(h, state):
                qn = in_pool.tile([128, NT * 64], BF16, tag="qn")
                kn = in_pool.tile([128, NT * 64], BF16, tag="kn")
                for qtr in range(4):
                    nc.gpsimd.dma_start(
                        qn[:].rearrange("p (t d) -> p t d", d=64)[:, 4*qtr:4*qtr+4, :],
                        q_d[h].rearrange("(t p) d -> p t d", p=128)[:, 4*qtr:4*qtr+4, :],
                    )
                for qtr in range(4):
                    nc.gpsimd.dma_start(
                        kn[:].rearrange("p (t d) -> p t d", d=64)[:, 4*qtr:4*qtr+4, :],
                        k_d[h].rearrange("(t p) d -> p t d", p=128)[:, 4*qtr:4*qtr+4, :],
                    )
                vp = v_pool.tile([128, NT * 65], BF16, tag="vp")
                vp3 = vp[:].rearrange("p (t e) -> p t e", e=65)
                nc.gpsimd.dma_start(
                    vp3[:, :, 0:64],
                    v_d[h].rearrange("(t p) d -> p t d", p=128),
                )
                nc.vector.tensor_copy(vp3[:, :, 64:65], ones16[:])
                state["qn"], state["kn"], state["vp"] = qn, kn, vp
                state["touch"] = [
                    pe_touch(nc, qn[0:1, 0:1]),
                    pe_touch(nc, kn[0:1, 0:1]),
                    pe_touch(nc, vp[0:1, 0:1]),
                    pe_touch(nc, vp[0:1, 64:65]),
                ]
                state["qt"] = qt_pool.tile([128, S], BF16, tag="qt", name="qt")
                state["kt"] = kt_pool.tile([128, 8 * 128], BF16, tag="kt", name="kt")
                state["first_tr"] = True

            def prep_q_group(g, state):
                qn, qt = state["qn"], state["qt"]
                stage = tr_ps.tile([128, 1024], BF16, tag="trstage")
                for s_i in range(4):
                    b = 4 * g + s_i
                    mm = nc.tensor.transpose(
                        stage[0:64, 128 * s_i:128 * s_i + 128],
                        qn[:, 64 * b:64 * b + 64],
                        ident_b[0:128, 0:128],
                    )
                    if state.pop("first_tr", False):
                        for t in (t_if, t_ir, *state["touch"]):
                            add_dep_helper(mm.ins, t.ins, sync=False,
                                           reason="presync")
                nc.vector.tensor_copy(
                    qt[0:64, 512 * g:512 * g + 512], stage[0:64, 0:512]
                )
                nc.sync.dma_start(
                    qt[64:128, 512 * g:512 * g + 512],
                    qt[0:64, 512 * g:512 * g + 512],
                )

            def prep_k_group(g, state):
                kn, kt = state["kn"], state["kt"]
                stage = tr_ps.tile([128, 1024], BF16, tag="trstage")
                for s_i in range(4):
                    t_i = 4 * g + s_i
                    nc.tensor.transpose(
                        stage[:, 128 * s_i:128 * s_i + 128],
                        kn[:, 128 * t_i:128 * t_i + 128],
                        ident_b[0:128, 0:128],
                    )
                nc.vector.tensor_copy(
                    kt[:, 512 * g:512 * g + 512], stage[:, 0:512]
                )

            def prep_finish(state):
                state["tq1"] = pe_touch(nc, state["qt"][0:1, 0:1])
                state["tk1"] = pe_touch(nc, state["kt"][0:1, 0:1])
                state["first_qk"] = True

            def prep_all(h, state):
                prep_loads(h, state)
                for g in range(4):
                    prep_q_group(g, state)
                for g in range(2):
                    prep_k_group(g, state)
                prep_finish(state)

            # ---------------- pipelined pair units -------------------------
            # unit = (h, c, t); per chunk c there are npair = 2c+2 pairs;
            # pair t covers k-tiles j1 = 2t, j2 = 2t+1. Diagonal pairs are
            # t == 2c (cA=0, cB=128) and t == 2c+1 (cA=256, cB=384).

            def emit_qk_exp(u, states):
                h, c, t = u
                state = states[h]
                qt, kt = state["qt"], state["kt"]
                j1, j2 = 2 * t, 2 * t + 1
                cA = 128 * j1 - 512 * c
                cB = 128 * j2 - 512 * c
                cA0 = max(0, cA)
                cB0 = max(0, cB)
                st = st_ps.tile([128, 1024], F32, tag="st")
                mmA = nc.tensor.matmul(
                    st[:, cA0:512],
                    kt[0:64, 128 * t:128 * t + 128],
                    qt[0:64, 512 * c + cA0:512 * c + 512],
                    start=True, stop=True,
                )
                if state.pop("first_qk", False):
                    for tch in (state["tq1"], state["tk1"]):
                        add_dep_helper(mmA.ins, tch.ins, sync=False,
                                       reason="presync")
                nc.tensor.matmul(
                    st[:, 512 + cB0:1024],
                    kt[64:128, 128 * t:128 * t + 128],
                    qt[64:128, 512 * c + cB0:512 * c + 512],
                    start=True, stop=True,
                )
                # one merged exp per pair; the [512:512+cB0] hole reads
                # stale PSUM whose exp lands in pt cols PV never touches.
                # exp waits ONLY on the QK matmuls: causal masking moves
                # post-exp onto the idle Pool engine (zero the upper
                # triangle of the diagonal pt blocks), keeping both DVE
                # and the pre-exp path out of the pair-latency loop.
                pt = pt_pool.tile([128, 1024], BF16, tag="pt")
                nc.scalar.activation(
                    pt[:, cA0:1024], st[:, cA0:1024],
                    mybir.ActivationFunctionType.Exp, scale=0.125,
                )
                if j1 >= 4 * c:
                    nc.gpsimd.affine_select(
                        out=pt[:, cA:cA + 128], in_=pt[:, cA:cA + 128],
                        compare_op=mybir.AluOpType.is_gt, fill=0.0, base=1,
                        pattern=[[1, 128]], channel_multiplier=-1,
                    )
                if j2 >= 4 * c:
                    nc.gpsimd.affine_select(
                        out=pt[:, 512 + cB:512 + cB + 128],
                        in_=pt[:, 512 + cB:512 + cB + 128],
                        compare_op=mybir.AluOpType.is_gt, fill=0.0, base=1,
                        pattern=[[1, 128]], channel_multiplier=-1,
                    )
                return {"pt": pt, "u": u}

            def emit_dead(h, c, t_dead, states):
                # zero-semaphore PE filler (same shape as the proven warmup
                # matmuls): keeps the HAM activity monitor warm while ACT
                # drains the exp backlog, so live matmuls stay at K=8/8
                for _ in range(2):
                    nc.tensor.matmul(
                        trash[:, 0:256],
                        ident_f[:, 0:64].bitcast(mybir.dt.bfloat16),
                        ident_f[:, 0:128].bitcast(mybir.dt.bfloat16),
                        start=True, stop=True, skip_group_check=True,
                    )

            def emit_pv(unit_data, states, ot_map):
                h, c, t = unit_data["u"]
                state = states[h]
                vp = state["vp"]
                pt = unit_data["pt"]
                npair = min(2 * c + 2, 8)
                j1, j2 = 2 * t, 2 * t + 1
                vA = max(0, 128 * j1 - 512 * c)
                vB = max(0, 128 * j2 - 512 * c)
                if t == 0:
                    ot_map[(h, c)] = ot_ps.tile(
                        [65, 512], F32, tag="ot", name="ot"
                    )
                ot = ot_map[(h, c)]
                nc.tensor.matmul(
                    ot[:, vA:512],
                    vp[:, 65 * j1:65 * j1 + 65],
                    pt[:, vA:512],
                    start=(t == 0), stop=False,
                    skip_group_check=True,
                )
                nc.tensor.matmul(
                    ot[:, vB:512],
                    vp[:, 65 * j2:65 * j2 + 65],
                    pt[:, 512 + vB:1024],
                    start=False, stop=(t == npair - 1),
                    skip_group_check=True,
                )

            def emit_tail(h, c, ot_map):
                ot = ot_map.pop((h, c))
                otsb = otsb_pool.tile([65, 512], F32R, tag="otsb")
                nc.vector.tensor_copy(otsb[:, :], ot[:, :])
                oq = ot_ps.tile([128, 4 * 96], F32R, tag="ot", name="oq")
                for i in range(4):
                    nc.tensor.transpose(
                        oq[:, 96 * i:96 * i + 96],
                        otsb[0:65, 128 * i:128 * i + 128],
                        ident_r[0:65, 0:96],
                    )
                oqv = oq[:].rearrange("p (i w) -> p i w", w=96)
                rec = r_pool.tile([128, 4], F32, tag="rec")
                nc.vector.reciprocal(
                    rec[:].rearrange("p (i o) -> p i o", o=1),
                    oqv[:, :, 64:65],
                )
                ob = out_pool.tile([128, 256], F32, tag="ob")
                for i in range(4):
                    nc.vector.tensor_scalar_mul(
                        ob[:, 64 * i:64 * i + 64],
                        oq[:, 96 * i:96 * i + 64],
                        rec[:, i:i + 1],
                    )
                nc.sync.dma_start(
                    o_d[h].rearrange("(t p) d -> p t d", p=128)[:, 4*c:4*c+4, :],
                    ob[:].rearrange("p (t d) -> p t d", d=64),
                )

            # Flat unit list with per-unit filler tasks (next head's prep).
            units = []
            for h in range(HEADS_PER_CORE):
                for c in range(NCHUNK):
                    for t in range(min(2 * c + 2, 8)):
                        units.append((h, c, t))
            # dead-QK filler counts per (c, t); last head's final chunk
            # gets synthetic fill (no real dead tiles left)
            def dead_plan(h, c):
                live = min(2 * c + 2, 8)
                fill = {0: 6, 1: 5, 2: 6, 3: 0}[c]
                if h == HEADS_PER_CORE - 1 and c == NCHUNK - 1:
                    fill = 4
                plan = [0] * live
                for d in range(fill):
                    plan[d % live] += 1
                return plan

            fillers = {}
            for h in range(HEADS_PER_CORE - 1):
                nh = h + 1
                fillers[(h, 2, 0)] = [lambda s, _h=nh: prep_loads(_h, s[_h])]
                slices = [
                    lambda s, _h=nh, g=g: prep_q_group(g, s[_h]) for g in range(4)
                ] + [
                    lambda s, _h=nh, g=g: prep_k_group(g, s[_h]) for g in range(2)
                ] + [lambda s, _h=nh: prep_finish(s[_h])]
                for t in range(7):
                    fillers[(h, 3, t)] = [slices[t]]

            states = {h: {} for h in range(HEADS_PER_CORE)}
            prep_all(0, states[0])

            ot_map = {}
            lag = []  # pending units awaiting PV emission (distance 1)
            for u in units:
                if len(lag) == 3:
                    ud = lag.pop(0)
                    emit_pv(ud, states, ot_map)
                    _h, _c, _t = ud["u"]
                    if _t == min(2 * _c + 2, 8) - 1:
                        emit_tail(_h, _c, ot_map)
                for f in fillers.get(u, []):
                    f(states)
                lag.append(emit_qk_exp(u, states))
                _h, _c, _t = u
                plan = dead_plan(_h, _c)
                live = len(plan)
                for d in range(plan[_t]):
                    nd = sum(plan[:_t]) + d
                    emit_dead(_h, _c, min(live + nd, 7), states)
            while lag:
                ud = lag.pop(0)
                emit_pv(ud, states, ot_map)
                _h, _c, _t = ud["u"]
                if _t == min(2 * _c + 2, 8) - 1:
                    emit_tail(_h, _c, ot_map)

    split_waits(nc)
    return nc


_CACHED = {}


def kernel(Q: np.ndarray, K: np.ndarray, V: np.ndarray) -> np.ndarray:
    res = _run(Q, K, V, trace=False)
    return res[0]


def _run(Q, K, V, trace=False):
    Qf = np.ascontiguousarray(Q.reshape(B * H, S, D), dtype=np.float32)
    Kf = np.ascontiguousarray(K.reshape(B * H, S, D), dtype=np.float32)
    Vf = np.ascontiguousarray(V.reshape(B * H, S, D), dtype=np.float32)

    in_maps = []
    for c in range(N_CORES):
        sl = slice(c * HEADS_PER_CORE, (c + 1) * HEADS_PER_CORE)
        in_maps.append({
            "Q": np.ascontiguousarray(Qf[sl]),
            "K": np.ascontiguousarray(Kf[sl]),
            "V": np.ascontiguousarray(Vf[sl]),
        })

    if "nc" not in _CACHED:
        _CACHED["nc"] = build_kernel()
    nc = _CACHED["nc"]

    res = run_bass_kernel_spmd(
        nc, in_maps, core_ids=list(range(N_CORES)), trace=trace
    )
    out = np.empty((B * H, S, D), dtype=np.float32)
    for c in range(N_CORES):
        out[c * HEADS_PER_CORE:(c + 1) * HEADS_PER_CORE] = res.results[c]["O"]
    return out.reshape(B, H, S, D), res


# revision 14
# speedup vs baseline: 1.9719x; 1.0372x over previous
"""Causal multi-head attention on 8 Trainium2 NeuronCores (Bass/Tile).

Problem: Q,K,V [B=2, h=16, S=2048, d=64] fp32; out = softmax(QK^T/8, causal) V.

Sharding: B*h = 32 heads split 4-per-core across 8 cores (head-parallel);
each core computes full causal attention for its 4 heads.

v2: software-pipelined pair loop. The v1 kernel emitted QK(t), mask, exp,
PV(t) per k-tile pair in program order, so the PE sat in a serialized
QK -> DVE mask -> ACT exp -> PV dependency loop (~2.2us/pair). v2 emits
PV(t) after QK(t+1) in the PE stream (distance-1 software pipeline), so
exp(t) on ACT overlaps QK(t+1) on PE and the pace is set by whichever
engine is busiest (ACT exp ~1.05us/pair) instead of the dependency loop.
Other changes: one merged exp per pair (stale PSUM holes are exp'd but
never read), prep(h+1) sliced across the last chunk of head h as PE
filler, batched O-tail (one oq tile, batched reciprocal, one out DMA).

Per-head schedule (all matmuls in float32r — full PE rate at N>=256):
  - Load Q,K natural [128, 16*64] via SWDGE cast fp32->fp32r; V' = [V | 1]
    per k-tile ([128, 16*65]).
  - PE transpose-mode (fp32r): Q -> Q^T [64, 2048] (+DMA row-dup to 64:128),
    K -> K^T stacked pairs [128, 8*128] (k-tile 2t on partitions 0:64,
    2t+1 on 64:128).
  - For each q-chunk c (512 cols) and live k-tile pair t:
      S^T pair [128k, 1024] in PSUM via two row-group matmuls
      (contraction d=64 on partition halves), causal -1e30 mask added on
      the diagonal blocks (DVE), one exp via ACT (scale=1/8 folded in)
      -> P^T [128, 1024] fp32r in SBUF.
      PV: O^T[65, 512] += V'_j^T @ P^T_j accumulated over j in PSUM
      (row 64 = softmax denominator l via the ones column).
  - Per chunk: O^T -> SBUF, 4 exact fp32 transposes via matmul-vs-identity
    into one PSUM tile, batched reciprocal of l, 4 scalar muls, one out DMA.
"""

import numpy as np

import concourse.bass as bass
import concourse.mybir as mybir
import concourse.tile as tile
from concourse.bass_utils import run_bass_kernel_spmd
from concourse.tile import add_dep_helper

N_CORES = 8
B, H, S, D = 2, 16, 2048, 64
HEADS_PER_CORE = (B * H) // N_CORES  # 4
NT = S // 128           # 16 k/q tiles per head
NCHUNK = S // 512       # 4 q-chunks per head
F32 = mybir.dt.float32
F32R = mybir.dt.float32r
BF16 = mybir.dt.bfloat16
NEG = -1.0e30


class SplitDrainTileContext(tile.TileContext):
    """TileContext whose tail drain splits its semaphore waits across
    single-wait SP nops — the TPB CTRL_NO struct holds one wait slot, so
    a drain waiting on >1 proc fails walrus codegen."""

    def _drain_and_barrier(self, tick_clock, wait_clock):
        import bass_rust
        from concourse.vector_clock import ScopedClock

        gc = tick_clock.global_clock
        for i, v in enumerate(list(gc)):
            if v <= 0:
                continue
            c = bass_rust.VectorClock()
            c.require_at_least(i, v)
            nop = self.nc.sync.nop(hint="preDrain", nofuse=True)
            wait_clock.add_sem_waits(nop.ins, ScopedClock({None: c}))
        drain_inst = self.nc.sync.drain()
        wait_clock.add_sem_waits(
            drain_inst.ins, ScopedClock({None: bass_rust.VectorClock()})
        )
        self.nc.all_engine_barrier()
        assert self.sems is not None
        popped = self.nc._tile_sem_poison_stack.pop()
        assert popped is self._sem_poison
        self.nc.clear_and_free_semaphores(list(self.sems.allocated().values()))
        self.nc.all_engine_barrier()


def pe_touch(nc, ap):
    """1-column bf16 ldweights reading `ap` — engine-level PE instruction
    that absorbs a producer's sync wait into the PE engine clock so that
    following 4-byte matmuls need at most one wait (walrus S3_LW limit)."""
    return nc.tensor.ldweights(ap.bitcast(mybir.dt.bfloat16))


def split_waits(nc):
    """Post-pass: every TPB instruction holds exactly ONE sync-wait slot;
    walrus codegen rejects more. Move extra waits onto inserted same-engine
    nofuse nops placed immediately before the instruction."""
    cnt = 0
    for fn in nc.m.functions:
        for bb in fn.blocks:
            lst = bb.instructions
            i = 0
            while i < len(lst):
                ins = lst[i]
                si = ins.sync_info
                if si is not None and si.on_wait and len(si.on_wait) > 1:
                    waits = list(si.on_wait)
                    for w in waits[:-1]:
                        nop = mybir.InstNoOp(name=f"wsplit_{cnt}", ins=[], outs=[])
                        cnt += 1
                        nop.engine = ins.engine
                        nop.bass_nofuse = True
                        nop.sync_info = mybir.SyncInfo(on_wait=[w], on_update=[])
                        lst.insert(i, nop)
                        i += 1
                    si.on_wait = [waits[-1]]
                i += 1
    return cnt


def build_kernel():
    nc = bass.Bass(trn_type="TRN2")
    q_d = nc.dram_tensor("Q", [HEADS_PER_CORE, S, D], F32, kind="ExternalInput")
    k_d = nc.dram_tensor("K", [HEADS_PER_CORE, S, D], F32, kind="ExternalInput")
    v_d = nc.dram_tensor("V", [HEADS_PER_CORE, S, D], F32, kind="ExternalInput")
    o_d = nc.dram_tensor("O", [HEADS_PER_CORE, S, D], F32, kind="ExternalOutput")

    with SplitDrainTileContext(nc) as tc:
        import contextlib

        with contextlib.ExitStack() as ctx:
            consts = ctx.enter_context(tc.tile_pool(name="consts", bufs=1))
            in_pool = ctx.enter_context(tc.tile_pool(name="in", bufs=3))
            v_pool = ctx.enter_context(tc.tile_pool(name="vp", bufs=2))
            qt_pool = ctx.enter_context(tc.tile_pool(name="qt", bufs=2))
            kt_pool = ctx.enter_context(tc.tile_pool(name="kt", bufs=2))
            pt_pool = ctx.enter_context(tc.tile_pool(name="pt", bufs=6))
            otsb_pool = ctx.enter_context(tc.tile_pool(name="otsb", bufs=3))
            out_pool = ctx.enter_context(tc.tile_pool(name="out", bufs=4))
            r_pool = ctx.enter_context(tc.tile_pool(name="recip", bufs=4))

            tr_ps = ctx.enter_context(tc.tile_pool(name="trps", bufs=1, space="PSUM"))
            trash_ps = ctx.enter_context(tc.tile_pool(name="trashps", bufs=1, space="PSUM"))
            st_ps = ctx.enter_context(tc.tile_pool(name="stps", bufs=2, space="PSUM"))
            ot_ps = ctx.enter_context(tc.tile_pool(name="otps", bufs=2, space="PSUM"))

            # constants
            ident_f = consts.tile([128, 128], F32, tag="ident_f")
            nc.gpsimd.memset(ident_f[:], 0.0)
            nc.gpsimd.affine_select(
                out=ident_f[:], in_=ident_f[:],
                compare_op=mybir.AluOpType.not_equal, fill=1.0, base=0,
                pattern=[[-1, 128]], channel_multiplier=1,
            )
            ident_r = consts.tile([128, 128], F32R, tag="ident_r")
            nc.vector.tensor_copy(ident_r[:], ident_f[:])
            ident_b = consts.tile([128, 128], BF16, tag="ident_b")
            nc.vector.tensor_copy(ident_b[:], ident_f[:])
            # causal mask for S^T [k_part, q_free] diag blocks:
            # keep 0 where q >= k (f >= p), NEG below
            nmask = consts.tile([128, 128], F32, tag="nmask")
            nc.gpsimd.memset(nmask[:], NEG)
            nc.gpsimd.affine_select(
                out=nmask[:], in_=nmask[:],
                compare_op=mybir.AluOpType.is_gt, fill=0.0, base=0,
                pattern=[[-1, 128]], channel_multiplier=1,
            )
            ones16 = consts.tile([128, NT], F32, tag="ones16")
            nc.gpsimd.memset(ones16[:], 1.0)
            t_if = pe_touch(nc, ident_f[0:1, 0:1])
            t_ir = pe_touch(nc, ident_r[0:1, 0:1])
            trash = trash_ps.tile([128, 512], F32, tag="trash")

            def emit_fill(n=1):
                # zero-semaphore PE filler: keeps the HAM activity monitor
                # warm while ACT drains the exp backlog
                for _ in range(2 * n):
                    nc.tensor.matmul(
                        trash[:, 0:256],
                        ident_f[:, 0:64].bitcast(mybir.dt.bfloat16),
                        ident_f[:, 0:128].bitcast(mybir.dt.bfloat16),
                        start=True, stop=True, skip_group_check=True,
                    )

            for _ in range(24):
                nc.tensor.matmul(
                    trash[:, 0:256],
                    ident_f[:, 0:64].bitcast(mybir.dt.bfloat16),
                    ident_f[:, 0:128].bitcast(mybir.dt.bfloat16),
                    start=True, stop=True, skip_group_check=True,
                )

            # ---------------- prep: loads + transposes, sliceable ----------
            def prep_loads(h, state):
                qn = in_pool.tile([128, NT * 64], BF16, tag="qn")
                kn = in_pool.tile([128, NT * 64], BF16, tag="kn")
                for qtr in range(4):
                    nc.gpsimd.dma_start(
                        qn[:].rearrange("p (t d) -> p t d", d=64)[:, 4*qtr:4*qtr+4, :],
                        q_d[h].rearrange("(t p) d -> p t d", p=128)[:, 4*qtr:4*qtr+4, :],
                    )
                    nc.gpsimd.dma_start(
                        kn[:].rearrange("p (t d) -> p t d", d=64)[:, 4*qtr:4*qtr+4, :],
                        k_d[h].rearrange("(t p) d -> p t d", p=128)[:, 4*qtr:4*qtr+4, :],
                    )
                vp = v_pool.tile([128, NT * 65], BF16, tag="vp")
                vp3 = vp[:].rearrange("p (t e) -> p t e", e=65)
                nc.gpsimd.dma_start(
                    vp3[:, :, 0:64],
                    v_d[h].rearrange("(t p) d -> p t d", p=128),
                )
                nc.vector.tensor_copy(vp3[:, :, 64:65], ones16[:])
                state["qn"], state["kn"], state["vp"] = qn, kn, vp
                state["touch"] = [
                    pe_touch(nc, qn[0:1, 0:1]),
                    pe_touch(nc, kn[0:1, 0:1]),
                    pe_touch(nc, vp[0:1, 0:1]),
                    pe_touch(nc, vp[0:1, 64:65]),
                ]
                state["qt"] = qt_pool.tile([128, S], BF16, tag="qt", name="qt")
                state["kt"] = kt_pool.tile([128, 8 * 128], BF16, tag="kt", name="kt")
                state["first_tr"] = True

            def prep_q_group(g, state):
                qn, qt = state["qn"], state["qt"]
                stage = tr_ps.tile([128, 1024], BF16, tag="trstage")
                for s_i in range(4):
                    b = 4 * g + s_i
                    mm = nc.tensor.transpose(
                        stage[0:64, 128 * s_i:128 * s_i + 128],
                        qn[:, 64 * b:64 * b + 64],
                        ident_b[0:128, 0:128],
                    )
                    if state.pop("first_tr", False):
                        for t in (t_if, t_ir, *state["touch"]):
                            add_dep_helper(mm.ins, t.ins, sync=False,
                                           reason="presync")
                nc.vector.tensor_copy(
                    qt[0:64, 512 * g:512 * g + 512], stage[0:64, 0:512]
                )
                nc.sync.dma_start(
                    qt[64:128, 512 * g:512 * g + 512],
                    qt[0:64, 512 * g:512 * g + 512],
                )
                emit_fill(1)

            def prep_k_group(g, state):
                kn, kt = state["kn"], state["kt"]
                stage = tr_ps.tile([128, 1024], BF16, tag="trstage")
                for s_i in range(4):
                    t_i = 4 * g + s_i
                    nc.tensor.transpose(
                        stage[:, 128 * s_i:128 * s_i + 128],
                        kn[:, 128 * t_i:128 * t_i + 128],
                        ident_b[0:128, 0:128],
                    )
                nc.vector.tensor_copy(
                    kt[:, 512 * g:512 * g + 512], stage[:, 0:512]
                )
                emit_fill(1)

            def prep_finish(state):
                state["tq1"] = pe_touch(nc, state["qt"][0:1, 0:1])
                state["tk1"] = pe_touch(nc, state["kt"][0:1, 0:1])
                state["first_qk"] = True

            def prep_head0_start(state):
                prep_loads(0, state)
                prep_q_group(0, state)
                prep_k_group(0, state)
                prep_finish(state)

            # ---------------- pipelined pair units -------------------------
            # unit = (h, c, t); per chunk c there are npair = 2c+2 pairs;
            # pair t covers k-tiles j1 = 2t, j2 = 2t+1. Diagonal pairs are
            # t == 2c (cA=0, cB=128) and t == 2c+1 (cA=256, cB=384).

            def emit_qk_exp(u, states):
                h, c, t = u
                state = states[h]
                qt, kt = state["qt"], state["kt"]
                j1, j2 = 2 * t, 2 * t + 1
                cA = 128 * j1 - 512 * c
                cB = 128 * j2 - 512 * c
                cA0 = max(0, cA)
                cB0 = max(0, cB)
                st = st_ps.tile([128, 1024], F32, tag="st")
                mmA = nc.tensor.matmul(
                    st[:, cA0:512],
                    kt[0:64, 128 * t:128 * t + 128],
                    qt[0:64, 512 * c + cA0:512 * c + 512],
                    start=True, stop=True,
                )
                if state.pop("first_qk", False):
                    for tch in (state["tq1"], state["tk1"]):
                        add_dep_helper(mmA.ins, tch.ins, sync=False,
                                       reason="presync")
                nc.tensor.matmul(
                    st[:, 512 + cB0:1024],
                    kt[64:128, 128 * t:128 * t + 128],
                    qt[64:128, 512 * c + cB0:512 * c + 512],
                    start=True, stop=True,
                )
                # one merged exp per pair; the [512:512+cB0] hole reads
                # stale PSUM whose exp lands in pt cols PV never touches.
                # exp waits ONLY on the QK matmuls: causal masking moves
                # post-exp onto the idle Pool engine (zero the upper
                # triangle of the diagonal pt blocks), keeping both DVE
                # and the pre-exp path out of the pair-latency loop.
                pt = pt_pool.tile([128, 1024], BF16, tag="pt")
                nc.scalar.activation(
                    pt[:, cA0:1024], st[:, cA0:1024],
                    mybir.ActivationFunctionType.Exp, scale=0.125,
                )
                if j1 >= 4 * c:
                    nc.gpsimd.affine_select(
                        out=pt[:, cA:cA + 128], in_=pt[:, cA:cA + 128],
                        compare_op=mybir.AluOpType.is_gt, fill=0.0, base=1,
                        pattern=[[1, 128]], channel_multiplier=-1,
                    )
                if j2 >= 4 * c:
                    nc.gpsimd.affine_select(
                        out=pt[:, 512 + cB:512 + cB + 128],
                        in_=pt[:, 512 + cB:512 + cB + 128],
                        compare_op=mybir.AluOpType.is_gt, fill=0.0, base=1,
                        pattern=[[1, 128]], channel_multiplier=-1,
                    )
                return {"pt": pt, "u": u}

            def emit_dead(h, c, t_dead, states):
                # zero-semaphore PE filler (same shape as the proven warmup
                # matmuls): keeps the HAM activity monitor warm while ACT
                # drains the exp backlog, so live matmuls stay at K=8/8
                for _ in range(2):
                    nc.tensor.matmul(
                        trash[:, 0:256],
                        ident_f[:, 0:64].bitcast(mybir.dt.bfloat16),
                        ident_f[:, 0:128].bitcast(mybir.dt.bfloat16),
                        start=True, stop=True, skip_group_check=True,
                    )

            def emit_pv(unit_data, states, ot_map):
                h, c, t = unit_data["u"]
                state = states[h]
                vp = state["vp"]
                pt = unit_data["pt"]
                npair = min(2 * c + 2, 8)
                j1, j2 = 2 * t, 2 * t + 1
                vA = max(0, 128 * j1 - 512 * c)
                vB = max(0, 128 * j2 - 512 * c)
                if t == 0:
                    ot_map[(h, c)] = ot_ps.tile(
                        [65, 512], F32, tag="ot", name="ot"
                    )
                ot = ot_map[(h, c)]
                nc.tensor.matmul(
                    ot[:, vA:512],
                    vp[:, 65 * j1:65 * j1 + 65],
                    pt[:, vA:512],
                    start=(t == 0), stop=False,
                    skip_group_check=True,
                )
                nc.tensor.matmul(
                    ot[:, vB:512],
                    vp[:, 65 * j2:65 * j2 + 65],
                    pt[:, 512 + vB:1024],
                    start=False, stop=(t == npair - 1),
                    skip_group_check=True,
                )

            def emit_tail(h, c, ot_map):
                ot = ot_map.pop((h, c))
                otsb = otsb_pool.tile([65, 512], F32R, tag="otsb")
                nc.vector.tensor_copy(otsb[:, :], ot[:, :])
                oq = ot_ps.tile([128, 4 * 96], F32R, tag="ot", name="oq")
                for i in range(4):
                    nc.tensor.transpose(
                        oq[:, 96 * i:96 * i + 96],
                        otsb[0:65, 128 * i:128 * i + 128],
                        ident_r[0:65, 0:96],
                    )
                oqv = oq[:].rearrange("p (i w) -> p i w", w=96)
                rec = r_pool.tile([128, 4], F32, tag="rec")
                nc.vector.reciprocal(
                    rec[:].rearrange("p (i o) -> p i o", o=1),
                    oqv[:, :, 64:65],
                )
                ob = out_pool.tile([128, 256], F32, tag="ob")
                for i in range(4):
                    nc.vector.tensor_scalar_mul(
                        ob[:, 64 * i:64 * i + 64],
                        oq[:, 96 * i:96 * i + 64],
                        rec[:, i:i + 1],
                    )
                nc.sync.dma_start(
                    o_d[h].rearrange("(t p) d -> p t d", p=128)[:, 4*c:4*c+4, :],
                    ob[:].rearrange("p (t d) -> p t d", d=64),
                )

            # Flat unit list with per-unit filler tasks (next head's prep).
            units = []
            for h in range(HEADS_PER_CORE):
                for c in range(NCHUNK):
                    for t in range(min(2 * c + 2, 8)):
                        units.append((h, c, t))
            # dead-QK filler counts per (c, t); last head's final chunk
            # gets synthetic fill (no real dead tiles left)
            def dead_plan(h, c):
                live = min(2 * c + 2, 8)
                fill = {0: 6, 1: 5, 2: 6, 3: 0}[c]
                if h == HEADS_PER_CORE - 1 and c == NCHUNK - 1:
                    fill = 4
                plan = [0] * live
                for d in range(fill):
                    plan[d % live] += 1
                return plan

            fillers = {
                (0, 0, 0): [lambda s: prep_q_group(1, s[0])],
                (0, 1, 0): [lambda s: prep_k_group(1, s[0])],
                (0, 1, 1): [lambda s: prep_q_group(2, s[0])],
                (0, 1, 2): [lambda s: prep_q_group(3, s[0])],
            }
            for h in range(HEADS_PER_CORE - 1):
                nh = h + 1
                fillers[(h, 2, 0)] = [lambda s, _h=nh: prep_loads(_h, s[_h])]
                slices = [
                    lambda s, _h=nh: prep_q_group(0, s[_h]),
                    lambda s, _h=nh: prep_k_group(0, s[_h]),
                    lambda s, _h=nh: prep_q_group(1, s[_h]),
                    lambda s, _h=nh: prep_k_group(1, s[_h]),
                    lambda s, _h=nh: prep_q_group(2, s[_h]),
                    lambda s, _h=nh: prep_q_group(3, s[_h]),
                    lambda s, _h=nh: prep_finish(s[_h]),
                ]
                for t in range(7):
                    fillers[(h, 3, t)] = [slices[t]]

            states = {h: {} for h in range(HEADS_PER_CORE)}
            prep_head0_start(states[0])

            ot_map = {}
            lag = []  # pending units awaiting PV emission (distance 1)
            for u in units:
                if len(lag) == 3:
                    ud = lag.pop(0)
                    emit_pv(ud, states, ot_map)
                    _h, _c, _t = ud["u"]
                    if _t == min(2 * _c + 2, 8) - 1:
                        emit_tail(_h, _c, ot_map)
                for f in fillers.get(u, []):
                    f(states)
                lag.append(emit_qk_exp(u, states))
                _h, _c, _t = u
                plan = dead_plan(_h, _c)
                live = len(plan)
                for d in range(plan[_t]):
                    nd = sum(plan[:_t]) + d
                    emit_dead(_h, _c, min(live + nd, 7), states)
            while lag:
                ud = lag.pop(0)
                emit_pv(ud, states, ot_map)
                _h, _c, _t = ud["u"]
                if _t == min(2 * _c + 2, 8) - 1:
                    emit_tail(_h, _c, ot_map)

    split_waits(nc)
    return nc


_CACHED = {}


def kernel(Q: np.ndarray, K: np.ndarray, V: np.ndarray) -> np.ndarray:
    res = _run(Q, K, V, trace=False)
    return res[0]


def _run(Q, K, V, trace=False):
    Qf = np.ascontiguousarray(Q.reshape(B * H, S, D), dtype=np.float32)
    Kf = np.ascontiguousarray(K.reshape(B * H, S, D), dtype=np.float32)
    Vf = np.ascontiguousarray(V.reshape(B * H, S, D), dtype=np.float32)

    in_maps = []
    for c in range(N_CORES):
        sl = slice(c * HEADS_PER_CORE, (c + 1) * HEADS_PER_CORE)
        in_maps.append({
            "Q": np.ascontiguousarray(Qf[sl]),
            "K": np.ascontiguousarray(Kf[sl]),
            "V": np.ascontiguousarray(Vf[sl]),
        })

    if "nc" not in _CACHED:
        _CACHED["nc"] = build_kernel()
    nc = _CACHED["nc"]

    res = run_bass_kernel_spmd(
        nc, in_maps, core_ids=list(range(N_CORES)), trace=trace
    )
    out = np.empty((B * H, S, D), dtype=np.float32)
    for c in range(N_CORES):
        out[c * HEADS_PER_CORE:(c + 1) * HEADS_PER_CORE] = res.results[c]["O"]
    return out.reshape(B, H, S, D), res


# revision 15
# speedup vs baseline: 1.9791x; 1.0037x over previous
"""Causal multi-head attention on 8 Trainium2 NeuronCores (Bass/Tile).

Problem: Q,K,V [B=2, h=16, S=2048, d=64] fp32; out = softmax(QK^T/8, causal) V.

Sharding: B*h = 32 heads split 4-per-core across 8 cores (head-parallel);
each core computes full causal attention for its 4 heads.

v2: software-pipelined pair loop. The v1 kernel emitted QK(t), mask, exp,
PV(t) per k-tile pair in program order, so the PE sat in a serialized
QK -> DVE mask -> ACT exp -> PV dependency loop (~2.2us/pair). v2 emits
PV(t) after QK(t+1) in the PE stream (distance-1 software pipeline), so
exp(t) on ACT overlaps QK(t+1) on PE and the pace is set by whichever
engine is busiest (ACT exp ~1.05us/pair) instead of the dependency loop.
Other changes: one merged exp per pair (stale PSUM holes are exp'd but
never read), prep(h+1) sliced across the last chunk of head h as PE
filler, batched O-tail (one oq tile, batched reciprocal, one out DMA).

Per-head schedule (all matmuls in float32r — full PE rate at N>=256):
  - Load Q,K natural [128, 16*64] via SWDGE cast fp32->fp32r; V' = [V | 1]
    per k-tile ([128, 16*65]).
  - PE transpose-mode (fp32r): Q -> Q^T [64, 2048] (+DMA row-dup to 64:128),
    K -> K^T stacked pairs [128, 8*128] (k-tile 2t on partitions 0:64,
    2t+1 on 64:128).
  - For each q-chunk c (512 cols) and live k-tile pair t:
      S^T pair [128k, 1024] in PSUM via two row-group matmuls
      (contraction d=64 on partition halves), causal -1e30 mask added on
      the diagonal blocks (DVE), one exp via ACT (scale=1/8 folded in)
      -> P^T [128, 1024] fp32r in SBUF.
      PV: O^T[65, 512] += V'_j^T @ P^T_j accumulated over j in PSUM
      (row 64 = softmax denominator l via the ones column).
  - Per chunk: O^T -> SBUF, 4 exact fp32 transposes via matmul-vs-identity
    into one PSUM tile, batched reciprocal of l, 4 scalar muls, one out DMA.
"""

import numpy as np

import concourse.bass as bass
import concourse.mybir as mybir
import concourse.tile as tile
from concourse.bass_utils import run_bass_kernel_spmd
from concourse.tile import add_dep_helper

N_CORES = 8
B, H, S, D = 2, 16, 2048, 64
HEADS_PER_CORE = (B * H) // N_CORES  # 4
NT = S // 128           # 16 k/q tiles per head
NCHUNK = S // 512       # 4 q-chunks per head
F32 = mybir.dt.float32
F32R = mybir.dt.float32r
BF16 = mybir.dt.bfloat16
NEG = -1.0e30


class SplitDrainTileContext(tile.TileContext):
    """TileContext whose tail drain splits its semaphore waits across
    single-wait SP nops — the TPB CTRL_NO struct holds one wait slot, so
    a drain waiting on >1 proc fails walrus codegen."""

    def _drain_and_barrier(self, tick_clock, wait_clock):
        import bass_rust
        from concourse.vector_clock import ScopedClock

        gc = tick_clock.global_clock
        for i, v in enumerate(list(gc)):
            if v <= 0:
                continue
            c = bass_rust.VectorClock()
            c.require_at_least(i, v)
            nop = self.nc.sync.nop(hint="preDrain", nofuse=True)
            wait_clock.add_sem_waits(nop.ins, ScopedClock({None: c}))
        drain_inst = self.nc.sync.drain()
        wait_clock.add_sem_waits(
            drain_inst.ins, ScopedClock({None: bass_rust.VectorClock()})
        )
        self.nc.all_engine_barrier()
        assert self.sems is not None
        popped = self.nc._tile_sem_poison_stack.pop()
        assert popped is self._sem_poison
        self.nc.clear_and_free_semaphores(list(self.sems.allocated().values()))
        self.nc.all_engine_barrier()


def pe_touch(nc, ap):
    """1-column bf16 ldweights reading `ap` — engine-level PE instruction
    that absorbs a producer's sync wait into the PE engine clock so that
    following 4-byte matmuls need at most one wait (walrus S3_LW limit)."""
    return nc.tensor.ldweights(ap.bitcast(mybir.dt.bfloat16))


def split_waits(nc):
    """Post-pass: every TPB instruction holds exactly ONE sync-wait slot;
    walrus codegen rejects more. Move extra waits onto inserted same-engine
    nofuse nops placed immediately before the instruction."""
    cnt = 0
    for fn in nc.m.functions:
        for bb in fn.blocks:
            lst = bb.instructions
            i = 0
            while i < len(lst):
                ins = lst[i]
                si = ins.sync_info
                if si is not None and si.on_wait and len(si.on_wait) > 1:
                    waits = list(si.on_wait)
                    for w in waits[:-1]:
                        nop = mybir.InstNoOp(name=f"wsplit_{cnt}", ins=[], outs=[])
                        cnt += 1
                        nop.engine = ins.engine
                        nop.bass_nofuse = True
                        nop.sync_info = mybir.SyncInfo(on_wait=[w], on_update=[])
                        lst.insert(i, nop)
                        i += 1
                    si.on_wait = [waits[-1]]
                i += 1
    return cnt


def build_kernel():
    nc = bass.Bass(trn_type="TRN2")
    q_d = nc.dram_tensor("Q", [HEADS_PER_CORE, S, D], F32, kind="ExternalInput")
    k_d = nc.dram_tensor("K", [HEADS_PER_CORE, S, D], F32, kind="ExternalInput")
    v_d = nc.dram_tensor("V", [HEADS_PER_CORE, S, D], F32, kind="ExternalInput")
    o_d = nc.dram_tensor("O", [HEADS_PER_CORE, S, D], F32, kind="ExternalOutput")

    with SplitDrainTileContext(nc) as tc:
        import contextlib

        with contextlib.ExitStack() as ctx:
            consts = ctx.enter_context(tc.tile_pool(name="consts", bufs=1))
            in_pool = ctx.enter_context(tc.tile_pool(name="in", bufs=3))
            v_pool = ctx.enter_context(tc.tile_pool(name="vp", bufs=2))
            qt_pool = ctx.enter_context(tc.tile_pool(name="qt", bufs=2))
            kt_pool = ctx.enter_context(tc.tile_pool(name="kt", bufs=2))
            pt_pool = ctx.enter_context(tc.tile_pool(name="pt", bufs=6))
            otsb_pool = ctx.enter_context(tc.tile_pool(name="otsb", bufs=3))
            out_pool = ctx.enter_context(tc.tile_pool(name="out", bufs=4))
            r_pool = ctx.enter_context(tc.tile_pool(name="recip", bufs=4))

            tr_ps = ctx.enter_context(tc.tile_pool(name="trps", bufs=1, space="PSUM"))
            trash_ps = ctx.enter_context(tc.tile_pool(name="trashps", bufs=1, space="PSUM"))
            st_ps = ctx.enter_context(tc.tile_pool(name="stps", bufs=2, space="PSUM"))
            ot_ps = ctx.enter_context(tc.tile_pool(name="otps", bufs=2, space="PSUM"))

            # constants
            ident_f = consts.tile([128, 128], F32, tag="ident_f")
            nc.gpsimd.memset(ident_f[:], 0.0)
            nc.gpsimd.affine_select(
                out=ident_f[:], in_=ident_f[:],
                compare_op=mybir.AluOpType.not_equal, fill=1.0, base=0,
                pattern=[[-1, 128]], channel_multiplier=1,
            )
            ident_r = consts.tile([128, 128], F32R, tag="ident_r")
            nc.vector.tensor_copy(ident_r[:], ident_f[:])
            ident_b = consts.tile([128, 128], BF16, tag="ident_b")
            nc.vector.tensor_copy(ident_b[:], ident_f[:])
            # causal mask for S^T [k_part, q_free] diag blocks:
            # keep 0 where q >= k (f >= p), NEG below
            nmask = consts.tile([128, 128], F32, tag="nmask")
            nc.gpsimd.memset(nmask[:], NEG)
            nc.gpsimd.affine_select(
                out=nmask[:], in_=nmask[:],
                compare_op=mybir.AluOpType.is_gt, fill=0.0, base=0,
                pattern=[[-1, 128]], channel_multiplier=1,
            )
            ones16 = consts.tile([128, NT], F32, tag="ones16")
            nc.gpsimd.memset(ones16[:], 1.0)
            t_if = pe_touch(nc, ident_f[0:1, 0:1])
            t_ir = pe_touch(nc, ident_r[0:1, 0:1])
            trash = trash_ps.tile([128, 512], F32, tag="trash")

            def emit_fill(n=1):
                # zero-semaphore PE filler: keeps the HAM activity monitor
                # warm while ACT drains the exp backlog
                for _ in range(2 * n):
                    nc.tensor.matmul(
                        trash[:, 0:256],
                        ident_f[:, 0:64].bitcast(mybir.dt.bfloat16),
                        ident_f[:, 0:128].bitcast(mybir.dt.bfloat16),
                        start=True, stop=True, skip_group_check=True,
                    )

            for _ in range(6):
                nc.tensor.matmul(
                    trash[:, 0:256],
                    ident_f[:, 0:64].bitcast(mybir.dt.bfloat16),
                    ident_f[:, 0:128].bitcast(mybir.dt.bfloat16),
                    start=True, stop=True, skip_group_check=True,
                )

            # ---------------- prep: loads + transposes, sliceable ----------
            def prep_loads(h, state):
                qn = in_pool.tile([128, NT * 64], BF16, tag="qn")
                kn = in_pool.tile([128, NT * 64], BF16, tag="kn")
                for qtr in range(4):
                    nc.gpsimd.dma_start(
                        qn[:].rearrange("p (t d) -> p t d", d=64)[:, 4*qtr:4*qtr+4, :],
                        q_d[h].rearrange("(t p) d -> p t d", p=128)[:, 4*qtr:4*qtr+4, :],
                    )
                    nc.gpsimd.dma_start(
                        kn[:].rearrange("p (t d) -> p t d", d=64)[:, 4*qtr:4*qtr+4, :],
                        k_d[h].rearrange("(t p) d -> p t d", p=128)[:, 4*qtr:4*qtr+4, :],
                    )
                vp = v_pool.tile([128, NT * 65], BF16, tag="vp")
                vp3 = vp[:].rearrange("p (t e) -> p t e", e=65)
                nc.gpsimd.dma_start(
                    vp3[:, :, 0:64],
                    v_d[h].rearrange("(t p) d -> p t d", p=128),
                )
                nc.vector.tensor_copy(vp3[:, :, 64:65], ones16[:])
                state["qn"], state["kn"], state["vp"] = qn, kn, vp
                state["touch"] = [
                    pe_touch(nc, qn[0:1, 0:1]),
                    pe_touch(nc, kn[0:1, 0:1]),
                    pe_touch(nc, vp[0:1, 0:1]),
                    pe_touch(nc, vp[0:1, 64:65]),
                ]
                state["qt"] = qt_pool.tile([128, S], BF16, tag="qt", name="qt")
                state["kt"] = kt_pool.tile([128, 8 * 128], BF16, tag="kt", name="kt")
                state["first_tr"] = True

            def prep_q_group(g, state):
                qn, qt = state["qn"], state["qt"]
                stage = tr_ps.tile([128, 1024], BF16, tag="trstage")
                for s_i in range(4):
                    b = 4 * g + s_i
                    mm = nc.tensor.transpose(
                        stage[0:64, 128 * s_i:128 * s_i + 128],
                        qn[:, 64 * b:64 * b + 64],
                        ident_b[0:128, 0:128],
                    )
                    if state.pop("first_tr", False):
                        for t in (t_if, t_ir, *state["touch"]):
                            add_dep_helper(mm.ins, t.ins, sync=False,
                                           reason="presync")
                nc.vector.tensor_copy(
                    qt[0:64, 512 * g:512 * g + 512], stage[0:64, 0:512]
                )
                nc.sync.dma_start(
                    qt[64:128, 512 * g:512 * g + 512],
                    qt[0:64, 512 * g:512 * g + 512],
                )
                emit_fill(1)

            def prep_k_group(g, state):
                kn, kt = state["kn"], state["kt"]
                stage = tr_ps.tile([128, 1024], BF16, tag="trstage")
                for s_i in range(4):
                    t_i = 4 * g + s_i
                    nc.tensor.transpose(
                        stage[:, 128 * s_i:128 * s_i + 128],
                        kn[:, 128 * t_i:128 * t_i + 128],
                        ident_b[0:128, 0:128],
                    )
                nc.vector.tensor_copy(
                    kt[:, 512 * g:512 * g + 512], stage[:, 0:512]
                )
                emit_fill(1)

            def prep_finish(state):
                state["tq1"] = pe_touch(nc, state["qt"][0:1, 0:1])
                state["tk1"] = pe_touch(nc, state["kt"][0:1, 0:1])
                state["first_qk"] = True

            def prep_head0_start(state):
                prep_loads(0, state)
                prep_q_group(0, state)
                prep_k_group(0, state)
                prep_finish(state)

            # ---------------- pipelined pair units -------------------------
            # unit = (h, c, t); per chunk c there are npair = 2c+2 pairs;
            # pair t covers k-tiles j1 = 2t, j2 = 2t+1. Diagonal pairs are
            # t == 2c (cA=0, cB=128) and t == 2c+1 (cA=256, cB=384).

            def emit_qk_exp(u, states):
                h, c, t = u
                state = states[h]
                qt, kt = state["qt"], state["kt"]
                j1, j2 = 2 * t, 2 * t + 1
                cA = 128 * j1 - 512 * c
                cB = 128 * j2 - 512 * c
                cA0 = max(0, cA)
                cB0 = max(0, cB)
                st = st_ps.tile([128, 1024], F32, tag="st")
                mmA = nc.tensor.matmul(
                    st[:, cA0:512],
                    kt[0:64, 128 * t:128 * t + 128],
                    qt[0:64, 512 * c + cA0:512 * c + 512],
                    start=True, stop=True,
                )
                if state.pop("first_qk", False):
                    for tch in (state["tq1"], state["tk1"]):
                        add_dep_helper(mmA.ins, tch.ins, sync=False,
                                       reason="presync")
                nc.tensor.matmul(
                    st[:, 512 + cB0:1024],
                    kt[64:128, 128 * t:128 * t + 128],
                    qt[64:128, 512 * c + cB0:512 * c + 512],
                    start=True, stop=True,
                )
                # one merged exp per pair; the [512:512+cB0] hole reads
                # stale PSUM whose exp lands in pt cols PV never touches.
                # exp waits ONLY on the QK matmuls: causal masking moves
                # post-exp onto the idle Pool engine (zero the upper
                # triangle of the diagonal pt blocks), keeping both DVE
                # and the pre-exp path out of the pair-latency loop.
                pt = pt_pool.tile([128, 1024], BF16, tag="pt")
                nc.scalar.activation(
                    pt[:, cA0:1024], st[:, cA0:1024],
                    mybir.ActivationFunctionType.Exp, scale=0.125,
                )
                if j1 >= 4 * c:
                    nc.gpsimd.affine_select(
                        out=pt[:, cA:cA + 128], in_=pt[:, cA:cA + 128],
                        compare_op=mybir.AluOpType.is_gt, fill=0.0, base=1,
                        pattern=[[1, 128]], channel_multiplier=-1,
                    )
                if j2 >= 4 * c:
                    nc.gpsimd.affine_select(
                        out=pt[:, 512 + cB:512 + cB + 128],
                        in_=pt[:, 512 + cB:512 + cB + 128],
                        compare_op=mybir.AluOpType.is_gt, fill=0.0, base=1,
                        pattern=[[1, 128]], channel_multiplier=-1,
                    )
                return {"pt": pt, "u": u}

            def emit_dead(h, c, t_dead, states):
                # zero-semaphore PE filler (same shape as the proven warmup
                # matmuls): keeps the HAM activity monitor warm while ACT
                # drains the exp backlog, so live matmuls stay at K=8/8
                for _ in range(2):
                    nc.tensor.matmul(
                        trash[:, 0:256],
                        ident_f[:, 0:64].bitcast(mybir.dt.bfloat16),
                        ident_f[:, 0:128].bitcast(mybir.dt.bfloat16),
                        start=True, stop=True, skip_group_check=True,
                    )

            def emit_pv(unit_data, states, ot_map):
                h, c, t = unit_data["u"]
                state = states[h]
                vp = state["vp"]
                pt = unit_data["pt"]
                npair = min(2 * c + 2, 8)
                j1, j2 = 2 * t, 2 * t + 1
                vA = max(0, 128 * j1 - 512 * c)
                vB = max(0, 128 * j2 - 512 * c)
                if t == 0:
                    ot_map[(h, c)] = ot_ps.tile(
                        [65, 512], F32, tag="ot", name="ot"
                    )
                ot = ot_map[(h, c)]
                nc.tensor.matmul(
                    ot[:, vA:512],
                    vp[:, 65 * j1:65 * j1 + 65],
                    pt[:, vA:512],
                    start=(t == 0), stop=False,
                    skip_group_check=True,
                )
                nc.tensor.matmul(
                    ot[:, vB:512],
                    vp[:, 65 * j2:65 * j2 + 65],
                    pt[:, 512 + vB:1024],
                    start=False, stop=(t == npair - 1),
                    skip_group_check=True,
                )

            def emit_tail(h, c, ot_map):
                ot = ot_map.pop((h, c))
                otsb = otsb_pool.tile([65, 512], F32R, tag="otsb")
                nc.vector.tensor_copy(otsb[:, :], ot[:, :])
                oq = ot_ps.tile([128, 4 * 96], F32R, tag="ot", name="oq")
                for i in range(4):
                    nc.tensor.transpose(
                        oq[:, 96 * i:96 * i + 96],
                        otsb[0:65, 128 * i:128 * i + 128],
                        ident_r[0:65, 0:96],
                    )
                oqv = oq[:].rearrange("p (i w) -> p i w", w=96)
                rec = r_pool.tile([128, 4], F32, tag="rec")
                nc.vector.reciprocal(
                    rec[:].rearrange("p (i o) -> p i o", o=1),
                    oqv[:, :, 64:65],
                )
                ob = out_pool.tile([128, 256], F32, tag="ob")
                for i in range(4):
                    nc.vector.tensor_scalar_mul(
                        ob[:, 64 * i:64 * i + 64],
                        oq[:, 96 * i:96 * i + 64],
                        rec[:, i:i + 1],
                    )
                nc.sync.dma_start(
                    o_d[h].rearrange("(t p) d -> p t d", p=128)[:, 4*c:4*c+4, :],
                    ob[:].rearrange("p (t d) -> p t d", d=64),
                )

            # Flat unit list with per-unit filler tasks (next head's prep).
            units = []
            for h in range(HEADS_PER_CORE):
                for c in range(NCHUNK):
                    for t in range(min(2 * c + 2, 8)):
                        units.append((h, c, t))
            # dead-QK filler counts per (c, t); last head's final chunk
            # gets synthetic fill (no real dead tiles left)
            def dead_plan(h, c):
                live = min(2 * c + 2, 8)
                fill = {0: 6, 1: 5, 2: 6, 3: 0}[c]
                if h == HEADS_PER_CORE - 1 and c == NCHUNK - 1:
                    fill = 4
                plan = [0] * live
                for d in range(fill):
                    plan[d % live] += 1
                return plan

            fillers = {
                (0, 0, 0): [lambda s: prep_q_group(1, s[0])],
                (0, 1, 0): [lambda s: prep_k_group(1, s[0])],
                (0, 1, 1): [lambda s: prep_q_group(2, s[0])],
                (0, 1, 2): [lambda s: prep_q_group(3, s[0])],
            }
            for h in range(HEADS_PER_CORE - 1):
                nh = h + 1
                fillers[(h, 2, 0)] = [lambda s, _h=nh: prep_loads(_h, s[_h])]
                slices = [
                    lambda s, _h=nh: prep_q_group(0, s[_h]),
                    lambda s, _h=nh: prep_k_group(0, s[_h]),
                    lambda s, _h=nh: prep_q_group(1, s[_h]),
                    lambda s, _h=nh: prep_k_group(1, s[_h]),
                    lambda s, _h=nh: prep_q_group(2, s[_h]),
                    lambda s, _h=nh: prep_q_group(3, s[_h]),
                    lambda s, _h=nh: prep_finish(s[_h]),
                ]
                for t in range(7):
                    fillers[(h, 3, t)] = [slices[t]]

            states = {h: {} for h in range(HEADS_PER_CORE)}
            prep_head0_start(states[0])

            ot_map = {}
            lag = []  # pending units awaiting PV emission (distance 1)
            for u in units:
                if len(lag) == 3:
                    ud = lag.pop(0)
                    emit_pv(ud, states, ot_map)
                    _h, _c, _t = ud["u"]
                    if _t == min(2 * _c + 2, 8) - 1:
                        emit_tail(_h, _c, ot_map)
                for f in fillers.get(u, []):
                    f(states)
                lag.append(emit_qk_exp(u, states))
                _h, _c, _t = u
                plan = dead_plan(_h, _c)
                live = len(plan)
                for d in range(plan[_t]):
                    nd = sum(plan[:_t]) + d
                    emit_dead(_h, _c, min(live + nd, 7), states)
            while lag:
                ud = lag.pop(0)
                emit_pv(ud, states, ot_map)
                _h, _c, _t = ud["u"]
                if _t == min(2 * _c + 2, 8) - 1:
                    emit_tail(_h, _c, ot_map)

    split_waits(nc)
    return nc


_CACHED = {}


def kernel(Q: np.ndarray, K: np.ndarray, V: np.ndarray) -> np.ndarray:
    res = _run(Q, K, V, trace=False)
    return res[0]


def _run(Q, K, V, trace=False):
    Qf = np.ascontiguousarray(Q.reshape(B * H, S, D), dtype=np.float32)
    Kf = np.ascontiguousarray(K.reshape(B * H, S, D), dtype=np.float32)
    Vf = np.ascontiguousarray(V.reshape(B * H, S, D), dtype=np.float32)

    in_maps = []
    for c in range(N_CORES):
        sl = slice(c * HEADS_PER_CORE, (c + 1) * HEADS_PER_CORE)
        in_maps.append({
            "Q": np.ascontiguousarray(Qf[sl]),
            "K": np.ascontiguousarray(Kf[sl]),
            "V": np.ascontiguousarray(Vf[sl]),
        })

    if "nc" not in _CACHED:
        _CACHED["nc"] = build_kernel()
    nc = _CACHED["nc"]

    res = run_bass_kernel_spmd(
        nc, in_maps, core_ids=list(range(N_CORES)), trace=trace
    )
    out = np.empty((B * H, S, D), dtype=np.float32)
    for c in range(N_CORES):
        out[c * HEADS_PER_CORE:(c + 1) * HEADS_PER_CORE] = res.results[c]["O"]
    return out.reshape(B, H, S, D), res
